# revision 45
# baseline (speedup 1.0000x reference)
"""Trainium2 distributed kernel for nn_AdMatcher (retrieval_knn).

Strategy (8 NeuronCores, SPMD):
  - index_embs sharded row-wise; scoring matmul in float32r (TF32-grade,
    1 cyc/col) -> chunked per-1024-col top-8 via DVE max/max_index.
  - AllToAll exchanges (val, idx) candidate pairs; each core extracts the
    approx global top-128 for its 64 queries (iterative max8/match_replace).
  - Winner (val, idx) pairs and embedding rows gathered via SWDGE indirect
    DMA (one offset per partition); exact fp32 rescore on PE + resort gives
    the exact top-128 order/values; idx permuted through a DRAM scratch.
  - Rerank cross-attention + MLP runs in bf16, query-parallel, with the
    kh projection folded into the query side (G = M_k^T @ QhS).

kernel(**inputs) returns (top_idx [512,128] i32, top_scores [512,128] f32,
rerank_scores [512] f32).
"""

import math
import numpy as np

from concourse import bass, bacc, mybir, tile

FP32 = mybir.dt.float32
FP32R = mybir.dt.float32r
BF16 = mybir.dt.bfloat16
U32 = mybir.dt.uint32
I32 = mybir.dt.int32

NEG = -1.0e30
ID = mybir.ActivationFunctionType.Identity
EXP = mybir.ActivationFunctionType.Exp
RELU = mybir.ActivationFunctionType.Relu


def _p(**kw):
    class P:
        pass

    p = P()
    for k, v in kw.items():
        setattr(p, k, v)
    return p


def make_params(B=512, D=256, N=200000, K=128, ncores=8, chunk=1024):
    nshard = N // ncores
    nch = math.ceil(nshard / chunk)
    return _p(
        B=B, D=D, N=N, K=K, ncores=ncores, chunk=chunk,
        nshard=nshard, nch=nch, npad=nch * chunk, qpc=B // ncores,
        cand=nch * 8, mw=ncores * nch * 8,
        nqb=max(B // 128, 1), qb_size=min(B, 128),
        nheads=8, hd=D // 8, dh=D // 2,
    )


def fp32r_round(a):
    """Host-side float32r rounding (round-to-nearest, 11-bit mantissa)."""
    b = np.ascontiguousarray(a, np.float32).view(np.uint32)
    r = (b + np.uint32(0x800)) & np.uint32(0xFFFFF000)
    return r.view(np.float32)


# ---------------------------------------------------------------------------
# Builder
# ---------------------------------------------------------------------------

def build_nc(p):
    nc = bacc.Bacc("TRN2", target_bir_lowering=False, debug=False,
                   num_devices=p.ncores)
    dt = FP32
    KD = 2
    QB, NQB = p.qb_size, p.nqb
    CH, NCH, CAND = p.chunk, p.nch, p.cand
    QPC, MW = p.qpc, p.mw
    ROWW = 2 * CAND

    def din(name, shape, dtype=dt):
        return nc.dram_tensor(name, shape, dtype, kind="ExternalInput")

    qT = din("qT", [128, KD, p.B], FP32R)
    qT_mine = din("qT_mine", [128, KD, QPC])
    eT = din("eT", [128, KD, p.npad], FP32R)
    eFull = din("eFull", [p.N, p.D])
    base_add = din("base_add", [128, CAND], U32)
    qbaseT = din("qbaseT", [128, QPC])       # i*CAND
    qiota = din("qiota", [128, QPC])         # i
    hmask = din("hmask", [128, KD, 8])
    ident = din("ident", [128, 128])
    Wcd = din("Wcd", [128, KD, KD, 128])
    Wk = din("Wk", [128, KD, KD, 128])       # M_k needs untransposed chunks
    WkT = din("WkT", [128, KD, p.D])
    WvT = din("WvT", [128, KD, p.D])
    WcqT = din("WcqT", [128, KD, p.D])
    WqT = din("WqT", [128, KD, p.D])
    WoT = din("WoT", [128, KD, p.D])
    Ws1T = din("Ws1T", [128, KD, p.dh])
    Ws2T = din("Ws2T", [128, 1])
    b_cq = din("b_cq", [128, KD, 1])
    b_cd = din("b_cd", [128, KD, 1])
    b_q = din("b_q", [128, KD, 1])
    b_k = din("b_k", [128, KD, 1])
    b_v = din("b_v", [128, KD, 1])
    b_o = din("b_o", [128, KD, 1])
    b_s1 = din("b_s1", [p.dh, 1])
    b_s2 = din("b_s2", [1, 1])

    out_idx = nc.dram_tensor("top_idx", [QPC, p.K], I32, kind="ExternalOutput")
    out_scores = nc.dram_tensor("top_scores", [QPC, p.K], dt,
                                kind="ExternalOutput")
    out_rr = nc.dram_tensor("rerank", [1, QPC], dt, kind="ExternalOutput")

    a2a_in = nc.dram_tensor("a2a_in", [p.B, ROWW], dt)
    a2a_out = nc.dram_tensor("a2a_out", [p.B, ROWW], dt)
    idx_scratch = nc.dram_tensor("idx_scratch", [p.K * QPC, 64], U32)

    with tile.TileContext(nc) as tc:
        # ================= Stage A: scoring + per-chunk top-8 =============
        with tc.tile_pool(name="qt", bufs=1) as qt_pool, \
             tc.tile_pool(name="et", bufs=3) as et_pool, \
             tc.tile_pool(name="psA", bufs=3, space="PSUM") as psA, \
             tc.tile_pool(name="candp", bufs=1) as cand_pool, \
             tc.tile_pool(name="miscA", bufs=1) as miscA:

            qT_sb = qt_pool.tile([128, KD, p.B], FP32R)
            nc.sync.dma_start(out=qT_sb[:], in_=qT[:])
            base_sb = miscA.tile([128, CAND], U32)
            nc.sync.dma_start(out=base_sb[:], in_=base_add[:])

            packs = [cand_pool.tile([QB, ROWW], U32, tag=f"pack{qb}",
                                    name=f"pack{qb}")
                     for qb in range(NQB)]

            for c in range(NCH):
                et_tile = et_pool.tile([128, KD, CH], FP32R, tag="et")
                nc.sync.dma_start(out=et_tile[:],
                                  in_=eT[:, :, c * CH:(c + 1) * CH])
                for qb in range(NQB):
                    ps = psA.tile([128, CH], FP32, tag="psA")
                    mmw = min(CH, 512)
                    for kc in range(KD):
                        for h in range(CH // mmw):
                            nc.tensor.matmul(
                                out=ps[:QB, h * mmw:(h + 1) * mmw],
                                lhsT=qT_sb[:, kc, qb * QB:(qb + 1) * QB],
                                rhs=et_tile[:, kc, h * mmw:(h + 1) * mmw],
                                start=(kc == 0), stop=(kc == KD - 1))
                    pv = packs[qb][:].rearrange("q (c t) -> q c t", t=2)
                    vals8 = pv[:, c * 8:(c + 1) * 8, 0].bitcast(FP32)
                    nc.vector.max(out=vals8, in_=ps[:QB, :])
                    nc.vector.max_index(
                        out=pv[:, c * 8:(c + 1) * 8, 1],
                        in_max=vals8, in_values=ps[:QB, :])
            for qb in range(NQB):
                pv = packs[qb][:].rearrange("q (c t) -> q c t", t=2)
                nc.vector.tensor_tensor(
                    out=pv[:, :, 1], in0=pv[:, :, 1],
                    in1=base_sb[:QB, :], op=mybir.AluOpType.add)
                nc.sync.dma_start(
                    out=a2a_in[qb * QB:(qb + 1) * QB, :].bitcast(U32),
                    in_=packs[qb][:])

            nc.gpsimd.collective_compute(
                "AllToAll", mybir.AluOpType.bypass,
                replica_groups=[list(range(p.ncores))],
                ins=[a2a_in[:].opt()], outs=[a2a_out[:].opt()])

        # ================= Stage C: global merge ==========================
        with tc.tile_pool(name="mrg", bufs=1) as mrg_pool, \
             tc.tile_pool(name="sel", bufs=1) as sel_pool, \
             tc.tile_pool(name="pst", bufs=1, space="PSUM") as pst:
            # contiguous load of all (val, idx) pairs for my queries;
            # the extraction scans the stride-2 value view in place.
            pairs_sb = mrg_pool.tile([QPC, p.ncores, 2 * CAND], dt)
            nc.sync.dma_start(
                out=pairs_sb[:],
                in_=a2a_out.ap().rearrange("(s i) w -> i s w", s=p.ncores))
            vals_sb = mrg_pool.tile([QPC, MW], dt)
            nc.vector.tensor_copy(
                out=vals_sb[:].rearrange("i (s w) -> i s w", s=p.ncores),
                in_=pairs_sb[:].rearrange("i s (w t) -> i s w t",
                                          t=2)[:, :, :, 0])
            work = vals_sb[:]

            appr_sb = sel_pool.tile([QPC, p.K], dt)
            slots_sb = sel_pool.tile([QPC, p.K], U32)
            nrounds = p.K // 8
            for r in range(nrounds):
                v8 = appr_sb[:, r * 8:(r + 1) * 8]
                nc.vector.max(out=v8, in_=work)
                nc.vector.max_index(out=slots_sb[:, r * 8:(r + 1) * 8],
                                    in_max=v8, in_values=work)
                if r != nrounds - 1:
                    nc.vector.match_replace(out=work, in_to_replace=v8,
                                            in_values=work, imm_value=NEG)

            # ---- slot -> pair index (rank-on-partition orientation) ----
            inv = np.float32(1.0) / np.float32(CAND)
            MAGIC = np.float32(12582912.0)
            FEPS = np.float32(0.4985)
            fl = np.float32(np.float32(np.float32(
                np.arange(MW, dtype=np.float32) * inv) - FEPS) + MAGIC) - MAGIC
            assert np.all(fl == (np.arange(MW) // CAND)), "floor trick invalid"

            id_sb = sel_pool.tile([128, 128], dt)
            nc.sync.dma_start(out=id_sb[:], in_=ident[:])
            qb_sb = sel_pool.tile([128, QPC], dt)
            nc.sync.dma_start(out=qb_sb[:], in_=qbaseT[:])
            sf = sel_pool.tile([QPC, p.K], dt)
            nc.vector.tensor_copy(out=sf[:], in_=slots_sb[:])
            tp = pst.tile([128, 128], FP32, tag="tp")
            nc.tensor.transpose(out=tp[:, :QPC], in_=sf[:],
                                identity=id_sb[:QPC, :QPC])
            sfT = sel_pool.tile([p.K, QPC], dt)
            nc.scalar.activation(out=sfT[:], in_=tp[:p.K, :QPC], func=ID)
            segf = sel_pool.tile([p.K, QPC], dt)
            nc.vector.tensor_scalar(out=segf[:], in0=sfT[:],
                                    scalar1=float(inv), scalar2=float(FEPS),
                                    op0=mybir.AluOpType.mult,
                                    op1=mybir.AluOpType.subtract)
            nc.vector.tensor_scalar(out=segf[:], in0=segf[:],
                                    scalar1=float(MAGIC), scalar2=float(MAGIC),
                                    op0=mybir.AluOpType.add,
                                    op1=mybir.AluOpType.subtract)
            nc.vector.tensor_scalar(out=segf[:], in0=segf[:],
                                    scalar1=float(CAND * (QPC - 1)),
                                    scalar2=None, op0=mybir.AluOpType.mult)
            nc.vector.tensor_add(sfT[:], sfT[:], segf[:])
            nc.vector.tensor_tensor(out=sfT[:], in0=sfT[:],
                                    in1=qb_sb[:p.K, :],
                                    op=mybir.AluOpType.add)
            flat_u = sel_pool.tile([p.K, QPC], U32)
            nc.vector.tensor_copy(out=flat_u[:], in_=sfT[:])

            # ---- per-query gathers: (val, idx) pair then embedding row ----
            a2a_pairs = a2a_out.ap().rearrange("b (w t) -> (b w) t", t=2)
            pairbuf = sel_pool.tile([p.K, QPC, 2], dt)
            cand_all = sel_pool.tile([p.K, QPC, p.D], dt)
            for i in range(QPC):
                nc.gpsimd.indirect_dma_start(
                    out=pairbuf[:, i, :], out_offset=None,
                    in_=a2a_pairs,
                    in_offset=bass.IndirectOffsetOnAxis(
                        ap=flat_u[:, i:i + 1], axis=0))
                nc.gpsimd.indirect_dma_start(
                    out=cand_all[:, i, :], out_offset=None,
                    in_=eFull[:],
                    in_offset=bass.IndirectOffsetOnAxis(
                        ap=pairbuf[:, i, 1:2].bitcast(U32), axis=0))

            # stash gathered ids for the final permutation (col 0 of each
            # 256-byte row so dma_gather's row-size constraint is met)
            nc.sync.dma_start(
                out=idx_scratch[:, 0:1],
                in_=pairbuf[:, :, 1].bitcast(U32).unsqueeze(2))

            _rerank(nc, tc, p, cand_all, pairbuf, idx_scratch, qT_mine,
                    WcqT, WqT, Wcd, Wk, WkT, WvT, WoT, Ws1T, Ws2T, b_cq,
                    b_cd, b_q, b_k, b_v, b_o, b_s1, b_s2, hmask, qiota,
                    id_sb, pst, out_idx, out_scores, out_rr)

    nc.compile()
    return nc


def _rerank(nc, tc, p, cand_all, pairbuf, idx_scratch, qT_mine, WcqT, WqT,
            Wcd, Wk, WkT, WvT, WoT, Ws1T, Ws2T, b_cq, b_cd, b_q, b_k, b_v,
            b_o, b_s1, b_s2, hmask, qiota, id_sb, pst, out_idx, out_scores,
            out_rr):
    dt = FP32
    KD = 2
    QPC = p.qpc

    with tc.tile_pool(name="wts", bufs=1) as wp, \
         tc.tile_pool(name="cT", bufs=6) as cT_pool, \
         tc.tile_pool(name="kvV", bufs=17) as kvV, \
         tc.tile_pool(name="at", bufs=4) as at_pool, \
         tc.tile_pool(name="ctxp", bufs=1) as ctx_pool, \
         tc.tile_pool(name="psB", bufs=3, space="PSUM") as psB, \
         tc.tile_pool(name="psSm", bufs=2, space="PSUM") as psSm:

        def big_ps():
            return psB.tile([128, 512], FP32, tag="big", name="bigps")

        def ldb(t, shape):
            """Load a weight tensor, casting fp32 -> bf16 in the DMA."""
            s = wp.tile(shape, BF16, tag=t.name, name="w_" + t.name)
            nc.gpsimd.dma_start(out=s[:], in_=t[:])
            return s

        def ldf(t, shape):
            s = wp.tile(shape, dt, tag=t.name, name="w_" + t.name)
            nc.sync.dma_start(out=s[:], in_=t[:])
            return s

        wcd = ldb(Wcd, [128, KD, KD, 128])
        wkT = ldb(WkT, [128, KD, p.D])
        wvT = ldb(WvT, [128, KD, p.D])
        wcqT = ldb(WcqT, [128, KD, p.D])
        wqT = ldb(WqT, [128, KD, p.D])
        woT = ldb(WoT, [128, KD, p.D])
        ws1T = ldb(Ws1T, [128, KD, p.dh])
        ws2T = ldb(Ws2T, [128, 1])
        bcq = ldf(b_cq, [128, KD, 1])
        bcd = ldf(b_cd, [128, KD, 1])
        bq = ldf(b_q, [128, KD, 1])
        bk = ldf(b_k, [128, KD, 1])
        bv = ldf(b_v, [128, KD, 1])
        bo = ldf(b_o, [128, KD, 1])
        bs1 = ldf(b_s1, [p.dh, 1])
        bs2 = ldf(b_s2, [1, 1])
        hm = ldb(hmask, [128, KD, 8])
        hmf = ldf(hmask, [128, KD, 8])
        qtm = ldf(qT_mine, [128, KD, QPC])
        qtm_b = wp.tile([128, KD, QPC], BF16, tag="qtmb")
        nc.vector.tensor_copy(out=qtm_b[:], in_=qtm[:])
        qio = ldf(qiota, [128, QPC])

        # M_vT (bf16): [128(j), jc, i] = (Wv @ Wcd)[i, jc*128+j]
        mvT = wp.tile([128, KD, p.D], BF16, tag="mvT")
        for jc in range(KD):
            ps = big_ps()
            for lc in range(KD):
                nc.tensor.matmul(out=ps[:, :p.D], lhsT=wcd[:, lc, jc, :],
                                 rhs=wvT[:, lc, :],
                                 start=(lc == 0), stop=(lc == KD - 1))
            nc.scalar.activation(out=mvT[:, jc, :], in_=ps[:, :p.D], func=ID)
        # M_k untransposed chunks (bf16): mk[128(j), jc(j), ic(j'), 128(j')]
        #   = M_k[jc*128+j, ic*128+j'] = sum_l Wk[j, l] Wcd[l, j']
        mk = wp.tile([128, KD, KD, 128], BF16, tag="mk")
        for jc in range(KD):
            ps = big_ps()
            for lc in range(KD):
                # out[j, j'] = sum_l WkT[l, jc*128+j]^T ... lhsT = wkT chunk
                nc.tensor.matmul(out=ps[:, :p.D],
                                 lhsT=wkT[:, lc, jc * 128:(jc + 1) * 128],
                                 rhs=wcd[:, lc, :, :].rearrange("l a b -> l (a b)"),
                                 start=(lc == 0), stop=(lc == KD - 1))
            nc.scalar.activation(
                out=mk[:, jc, :, :].rearrange("j a b -> j (a b)"),
                in_=ps[:, :p.D], func=ID)

        # c_k = Wk @ b_cd + bk ; c_v = Wv @ b_cd + bv
        bcd_b = wp.tile([128, KD, 1], BF16, tag="bcdb")
        nc.vector.tensor_copy(out=bcd_b[:], in_=bcd[:])
        ck = wp.tile([128, KD, 1], dt, tag="ck")
        cv = wp.tile([128, KD, 1], dt, tag="cv")
        for (ct, wt, bt) in ((ck, wkT, bk), (cv, wvT, bv)):
            for m in range(KD):
                ps = psSm.tile([128, 8], FP32, tag="small", name="smps")
                for jc in range(KD):
                    nc.tensor.matmul(out=ps[:, :1],
                                     lhsT=wt[:, jc, m * 128:(m + 1) * 128],
                                     rhs=bcd_b[:, jc, :],
                                     start=(jc == 0), stop=(jc == KD - 1))
                nc.scalar.activation(out=ct[:, m, :], in_=ps[:, :1], func=ID,
                                     bias=bt[:, m, :])

        # Q'T = W_cq @ qT_mine + b_cq ; qhT = (Wq @ Q'T + bq) / sqrt(hd)
        qpT = wp.tile([128, KD, QPC], BF16, tag="qpT")
        qhT = wp.tile([128, KD, QPC], dt, tag="qhT")
        for m in range(KD):
            ps = big_ps()
            for jc in range(KD):
                nc.tensor.matmul(out=ps[:, :QPC],
                                 lhsT=wcqT[:, jc, m * 128:(m + 1) * 128],
                                 rhs=qtm_b[:, jc, :], start=(jc == 0),
                                 stop=(jc == KD - 1))
            nc.scalar.activation(out=qpT[:, m, :], in_=ps[:, :QPC], func=ID,
                                 bias=bcq[:, m, :])
        sc = 1.0 / math.sqrt(p.hd)
        bq_s = wp.tile([128, KD, 1], dt, tag="bqs")
        nc.vector.tensor_scalar(out=bq_s[:], in0=bq[:], scalar1=sc,
                                scalar2=None, op0=mybir.AluOpType.mult)
        for m in range(KD):
            ps = big_ps()
            for jc in range(KD):
                nc.tensor.matmul(out=ps[:, :QPC],
                                 lhsT=wqT[:, jc, m * 128:(m + 1) * 128],
                                 rhs=qpT[:, jc, :], start=(jc == 0),
                                 stop=(jc == KD - 1))
            nc.scalar.activation(out=qhT[:, m, :], in_=ps[:, :QPC], func=ID,
                                 bias=bq_s[:, m, :], scale=sc)

        # QhS_all[j, jc, q, h] = qhT * hmask ; G = M_k^T @ QhS (bf16)
        qhs_all = wp.tile([128, KD, QPC, 8], BF16, tag="qhs")
        nc.vector.tensor_tensor(
            out=qhs_all[:],
            in0=qhT[:].unsqueeze(3).to_broadcast([128, KD, QPC, 8]),
            in1=hmf[:].unsqueeze(2).to_broadcast([128, KD, QPC, 8]),
            op=mybir.AluOpType.mult)
        g_all = wp.tile([128, KD, QPC, 8], BF16, tag="g_all")
        for jpc in range(KD):
            nmm = (QPC * 8) // 512
            for b in range(max(nmm, 1)):
                w = min(512, QPC * 8)
                ps = big_ps()
                for jc in range(KD):
                    nc.tensor.matmul(
                        out=ps[:, :w],
                        lhsT=mk[:, jc, jpc, :],
                        rhs=qhs_all[:, jc, :, :].rearrange(
                            "j q h -> j (q h)")[:, b * 512:b * 512 + w],
                        start=(jc == 0), stop=(jc == KD - 1))
                nc.scalar.activation(
                    out=g_all[:, jpc, :, :].rearrange(
                        "j q h -> j (q h)")[:, b * 512:b * 512 + w],
                    in_=ps[:, :w], func=ID)

        # ---- per-query: transpose cand, rescore, attention ----
        sexT = ctx_pool.tile([p.K, QPC], dt)
        ctxcol = ctx_pool.tile([128, KD, QPC, 8], dt)
        nbat = max(QPC // 16, 1)
        bsz = QPC // nbat
        for bi in range(nbat):
            attn = at_pool.tile([8, bsz, 128], dt, tag="attn")
            vhs = []
            for qq in range(bsz):
                q = bi * bsz + qq
                candq = cand_all[:, q, :]
                cTf = cT_pool.tile([128, KD, 128], dt, tag="cTf")
                cTb = cT_pool.tile([128, KD, 128], BF16, tag="cTb")
                for m in range(KD):
                    tpp = big_ps()
                    nc.tensor.transpose(out=tpp[:, :128],
                                        in_=candq[:, m * 128:(m + 1) * 128],
                                        identity=id_sb[:])
                    nc.scalar.activation(out=cTf[:, m, :], in_=tpp[:, :128],
                                         func=ID)
                    nc.scalar.activation(out=cTb[:, m, :], in_=tpp[:, :128],
                                         func=ID)
                # exact rescore: s_ex[k] = sum_d candT[d,k] * q[d]
                ps = psSm.tile([128, 8], FP32, tag="small", name="smps2")
                for jc in range(KD):
                    nc.tensor.matmul(out=ps[:, :1], lhsT=cTf[:, jc, :],
                                     rhs=qtm[:, jc, q:q + 1],
                                     start=(jc == 0), stop=(jc == KD - 1))
                nc.scalar.activation(out=sexT[:, q:q + 1], in_=ps[:, :1],
                                     func=ID)
                # vh = cand @ M_v^T + (c_v deferred)  [128k, D] bf16
                vh = kvV.tile([128, p.D], BF16, tag="vh")
                ps2 = big_ps()
                for jc in range(KD):
                    nc.tensor.matmul(out=ps2[:, :p.D], lhsT=cTb[:, jc, :],
                                     rhs=mvT[:, jc, :],
                                     start=(jc == 0), stop=(jc == KD - 1))
                nc.scalar.activation(out=vh[:], in_=ps2[:, :p.D], func=ID)
                vhs.append(vh)
                # logits = G_q^T @ candT
                lg_ps = psSm.tile([8, 128], FP32, tag="lgps", name="lgps")
                for jpc in range(KD):
                    nc.tensor.matmul(out=lg_ps[:],
                                     lhsT=g_all[:, jpc, q, :],
                                     rhs=cTb[:, jpc, :],
                                     start=(jpc == 0), stop=(jpc == KD - 1))
                nc.scalar.activation(out=attn[:, qq, :], in_=lg_ps[:],
                                     func=ID)
            nmax = at_pool.tile([8, bsz], dt, tag="nmax")
            nc.vector.tensor_reduce(out=nmax[:], in_=attn[:],
                                    axis=mybir.AxisListType.X,
                                    op=mybir.AluOpType.max, negate=True)
            nc.vector.tensor_tensor(
                out=attn[:], in0=attn[:],
                in1=nmax[:].unsqueeze(2).to_broadcast([8, bsz, 128]),
                op=mybir.AluOpType.add)
            nc.scalar.activation(out=attn[:], in_=attn[:], func=EXP)
            ssum = at_pool.tile([8, bsz], dt, tag="ssum")
            nc.vector.tensor_reduce(out=ssum[:], in_=attn[:],
                                    axis=mybir.AxisListType.X,
                                    op=mybir.AluOpType.add)
            rinv = at_pool.tile([8, bsz], dt, tag="rinv")
            nc.vector.reciprocal(out=rinv[:], in_=ssum[:])
            nc.vector.tensor_tensor(
                out=attn[:], in0=attn[:],
                in1=rinv[:].unsqueeze(2).to_broadcast([8, bsz, 128]),
                op=mybir.AluOpType.mult)
            for qq in range(bsz):
                q = bi * bsz + qq
                tpp = psSm.tile([128, 8], FP32, tag="small", name="smps3")
                nc.tensor.transpose(out=tpp[:], in_=attn[:, qq, :],
                                    identity=id_sb[:8, :8])
                attnT = at_pool.tile([128, 8], BF16, tag="attnT")
                nc.scalar.activation(out=attnT[:], in_=tpp[:], func=ID)
                for m in range(KD):
                    ps3 = psSm.tile([128, 8], FP32, tag="small", name="smps4")
                    nc.tensor.matmul(out=ps3[:],
                                     lhsT=vhs[qq][:, m * 128:(m + 1) * 128],
                                     rhs=attnT[:], start=True, stop=True)
                    nc.scalar.activation(out=ctxcol[:, m, q, :], in_=ps3[:],
                                         func=ID, bias=cv[:, m, :])

        # ---- exact resort ----
        tp = pst.tile([128, 128], FP32, tag="tp")
        nc.tensor.transpose(out=tp[:QPC, :p.K], in_=sexT[:],
                            identity=id_sb[:])
        sex = ctx_pool.tile([QPC, p.K], dt)
        nc.scalar.activation(out=sex[:], in_=tp[:QPC, :p.K], func=ID)
        swork = ctx_pool.tile([QPC, p.K], dt)
        nc.vector.tensor_copy(out=swork[:], in_=sex[:])
        scr_sb = ctx_pool.tile([QPC, p.K], dt)
        slots3 = ctx_pool.tile([QPC, p.K], U32)
        nr = p.K // 8
        for r in range(nr):
            v8 = scr_sb[:, r * 8:(r + 1) * 8]
            nc.vector.max(out=v8, in_=swork[:])
            nc.vector.max_index(out=slots3[:, r * 8:(r + 1) * 8],
                                in_max=v8, in_values=swork[:])
            if r != nr - 1:
                nc.vector.match_replace(out=swork[:], in_to_replace=v8,
                                        in_values=swork[:], imm_value=NEG)
        nc.sync.dma_start(out=out_scores[:], in_=scr_sb[:])

        # ---- permute ids by exact rank via DRAM scratch ----
        s3f = ctx_pool.tile([QPC, p.K], dt)
        nc.vector.tensor_copy(out=s3f[:], in_=slots3[:])
        tp2 = pst.tile([128, 128], FP32, tag="tp")
        nc.tensor.transpose(out=tp2[:, :QPC], in_=s3f[:],
                            identity=id_sb[:QPC, :QPC])
        f3T = ctx_pool.tile([p.K, QPC], dt)
        # flat = slot*QPC + i
        nc.scalar.activation(out=f3T[:], in_=tp2[:p.K, :QPC], func=ID,
                             scale=float(QPC))
        nc.vector.tensor_tensor(out=f3T[:], in0=f3T[:], in1=qio[:p.K, :],
                                op=mybir.AluOpType.add)
        # batched permutation: one dma_gather over 256-byte scratch rows.
        # idx list order k = q*K + j -> wrapped[(j%16), q*(K//16) + j//16]
        f3i = ctx_pool.tile([p.K, QPC], mybir.dt.int16)
        nc.vector.tensor_copy(out=f3i[:], in_=f3T[:])
        wrapped = ctx_pool.tile([128, QPC * (p.K // 16)], mybir.dt.int16)
        wv = wrapped[:].rearrange("p (q h) -> p q h", h=p.K // 16)
        for hi in range(p.K // 16):
            nc.sync.dma_start(out=wv[:16, :, hi],
                              in_=f3i[hi * 16:(hi + 1) * 16, :])
        for g in range(1, 8):
            nc.sync.dma_start(out=wrapped[g * 16:(g + 1) * 16, :],
                              in_=wrapped[:16, :])
        half = (p.K * QPC) // 2
        idxff = ctx_pool.tile([p.K, QPC], dt)
        for hh in range(2):
            gbuf = ctx_pool.tile([128, half // 128, 64], dt, tag="gbuf",
                                 name=f"gbuf{hh}")
            nc.gpsimd.dma_gather(
                out_ap=gbuf[:], in_ap=idx_scratch.ap().bitcast(FP32),
                idxs_ap=wrapped[:, hh * (half // 16):(hh + 1) * (half // 16)],
                num_idxs=half, num_idxs_reg=half, elem_size=64,
                single_packet=False)
            nc.vector.tensor_copy(
                out=idxff[:].rearrange("j (hh q) -> j hh q",
                                       hh=2)[:, hh, :],
                in_=gbuf[:, :, 0].bitcast(U32))
        tp3 = pst.tile([128, 128], FP32, tag="tp")
        nc.tensor.transpose(out=tp3[:QPC, :p.K], in_=idxff[:],
                            identity=id_sb[:])
        idx_out_sb = ctx_pool.tile([QPC, p.K], I32)
        nc.vector.tensor_copy(out=idx_out_sb[:], in_=tp3[:QPC, :p.K])
        nc.sync.dma_start(out=out_idx[:], in_=idx_out_sb[:])


        # ---- diag extract + output head ----
        ctxTf = ctx_pool.tile([128, KD, QPC], dt)
        tmp = ctx_pool.tile([128, KD, QPC, 8], dt)
        nc.vector.tensor_tensor(
            out=tmp[:], in0=ctxcol[:],
            in1=hmf[:].unsqueeze(2).to_broadcast([128, KD, QPC, 8]),
            op=mybir.AluOpType.mult)
        nc.vector.tensor_reduce(out=ctxTf[:], in_=tmp[:],
                                axis=mybir.AxisListType.X,
                                op=mybir.AluOpType.add)
        ctxT = ctx_pool.tile([128, KD, QPC], BF16)
        nc.vector.tensor_copy(out=ctxT[:], in_=ctxTf[:])

        crossT = ctx_pool.tile([128, KD, QPC], BF16)
        for m in range(KD):
            ps = big_ps()
            for jc in range(KD):
                nc.tensor.matmul(out=ps[:, :QPC],
                                 lhsT=woT[:, jc, m * 128:(m + 1) * 128],
                                 rhs=ctxT[:, jc, :], start=(jc == 0),
                                 stop=(jc == KD - 1))
            nc.scalar.activation(out=crossT[:, m, :], in_=ps[:, :QPC],
                                 func=ID, bias=bo[:, m, :])
        s1T = ctx_pool.tile([p.dh, QPC], BF16)
        ps = big_ps()
        for jc in range(KD):
            nc.tensor.matmul(out=ps[:p.dh, :QPC], lhsT=ws1T[:, jc, :],
                             rhs=crossT[:, jc, :], start=(jc == 0),
                             stop=(jc == KD - 1))
        nc.scalar.activation(out=s1T[:], in_=ps[:p.dh, :QPC], func=RELU,
                             bias=bs1[:])
        rr_ps = big_ps()
        nc.tensor.matmul(out=rr_ps[:1, :QPC], lhsT=ws2T[:p.dh, :],
                         rhs=s1T[:], start=True, stop=True)
        rr_sb = ctx_pool.tile([1, QPC], dt)
        nc.scalar.activation(out=rr_sb[:], in_=rr_ps[:1, :QPC], func=ID,
                             bias=bs2[:])
        nc.sync.dma_start(out=out_rr[:], in_=rr_sb[:])


# ---------------------------------------------------------------------------
# Host-side glue
# ---------------------------------------------------------------------------

def _to2chunk(w):
    return np.ascontiguousarray(w.reshape(2, 128, -1).transpose(1, 0, 2))


def prepare_in_maps(p, inputs):
    f32 = np.float32
    q = np.asarray(inputs["query_emb"], f32)
    E = np.asarray(inputs["index_embs"], f32)
    W_cq = np.asarray(inputs["W_cq"], f32)
    b_cq = np.asarray(inputs["b_cq"], f32)
    W_cd = np.asarray(inputs["W_cd"], f32)
    b_cd = np.asarray(inputs["b_cd"], f32)
    ipw = np.asarray(inputs["in_proj_w"], f32)
    ipb = np.asarray(inputs["in_proj_b"], f32)
    Wq, Wk, Wv = ipw[:p.D], ipw[p.D:2 * p.D], ipw[2 * p.D:]
    bq, bk, bv = ipb[:p.D], ipb[p.D:2 * p.D], ipb[2 * p.D:]
    W_o = np.asarray(inputs["out_proj_w"], f32)
    b_o = np.asarray(inputs["out_proj_b"], f32)
    W_s1 = np.asarray(inputs["W_s1"], f32)
    b_s1 = np.asarray(inputs["b_s1"], f32)
    W_s2 = np.asarray(inputs["W_s2"], f32)
    b_s2 = np.asarray(inputs["b_s2"], f32)

    qT = _to2chunk(np.ascontiguousarray(q.T))
    heads = (np.arange(p.D) // p.hd)
    hmask = np.zeros((p.D, 8), f32)
    hmask[np.arange(p.D), heads] = 1.0
    hmask = _to2chunk(hmask)
    ident = np.eye(128, dtype=f32)
    wcd = np.ascontiguousarray(
        W_cd.reshape(2, 128, 2, 128).transpose(1, 0, 2, 3))
    wk = np.ascontiguousarray(
        Wk.reshape(2, 128, 2, 128).transpose(1, 0, 2, 3))

    def T2(w):
        return _to2chunk(np.ascontiguousarray(w.T))

    def bvec(b):
        return np.ascontiguousarray(b.reshape(2, 128, 1).transpose(1, 0, 2))

    common = dict(
        qT=fp32r_round(qT), eFull=E, hmask=hmask, ident=ident,
        Wcd=wcd, Wk=wk, WkT=T2(Wk), WvT=T2(Wv), WcqT=T2(W_cq), WqT=T2(Wq),
        WoT=T2(W_o), Ws1T=T2(W_s1), Ws2T=np.ascontiguousarray(W_s2.T),
        b_cq=bvec(b_cq), b_cd=bvec(b_cd), b_q=bvec(bq), b_k=bvec(bk),
        b_v=bvec(bv), b_o=bvec(b_o),
        b_s1=np.ascontiguousarray(b_s1.reshape(p.dh, 1)),
        b_s2=np.ascontiguousarray(b_s2.reshape(1, 1)),
    )
    in_maps = []
    for c in range(p.ncores):
        esh = E[c * p.nshard:(c + 1) * p.nshard]
        eTc = np.zeros((p.D, p.npad), f32)
        eTc[:, :p.nshard] = esh.T
        base = np.zeros((128, p.cand), np.uint32)
        for ch in range(p.nch):
            base[:, ch * 8:(ch + 1) * 8] = c * p.nshard + ch * p.chunk
        qbase = np.broadcast_to(
            (np.arange(p.qpc, dtype=np.float64) * p.cand).astype(f32),
            (128, p.qpc)).copy()
        qio = np.broadcast_to(
            np.arange(p.qpc, dtype=f32), (128, p.qpc)).copy()
        m = dict(common)
        m["eT"] = fp32r_round(_to2chunk(eTc))
        m["base_add"] = base
        m["qT_mine"] = np.ascontiguousarray(qT[:, :, c * p.qpc:(c + 1) * p.qpc])
        m["qbaseT"] = qbase
        m["qiota"] = qio
        in_maps.append(m)
    return in_maps


_CACHE = {}


def _get_nc(p):
    key = (p.B, p.N, p.chunk)
    if key not in _CACHE:
        _CACHE[key] = build_nc(p)
    return _CACHE[key]


def run(inputs, trace=False, **kw):
    from concourse.bass_utils import run_bass_kernel_spmd
    p = make_params()
    nc = _get_nc(p)
    in_maps = prepare_in_maps(p, inputs)
    res = run_bass_kernel_spmd(nc, in_maps, core_ids=list(range(p.ncores)),
                               trace=trace, **kw)
    outs = res.results
    top_idx = np.concatenate([o["top_idx"] for o in outs], axis=0)
    top_scores = np.concatenate([o["top_scores"] for o in outs], axis=0)
    rerank = np.concatenate([o["rerank"].reshape(-1) for o in outs], axis=0)
    return (top_idx.astype(np.int32), top_scores, rerank), res


def kernel(**inputs):
    out, _ = run(inputs, trace=False)
    return out


# revision 46
# speedup vs baseline: 1.1479x; 1.1479x over previous
"""Trainium2 distributed kernel for nn_AdMatcher (retrieval_knn).

Strategy (8 NeuronCores, SPMD):
  - index_embs sharded row-wise; scoring matmul in float32r (TF32-grade,
    1 cyc/col) -> chunked per-1024-col top-8 via DVE max/max_index.
  - AllToAll exchanges (val, idx) candidate pairs; each core extracts the
    approx global top-128 for its 64 queries (iterative max8/match_replace).
  - Winner (val, idx) pairs and embedding rows gathered via SWDGE indirect
    DMA (one offset per partition); exact fp32 rescore on PE + resort gives
    the exact top-128 order/values; idx permuted through a DRAM scratch.
  - Rerank cross-attention + MLP runs in bf16, query-parallel, with the
    kh projection folded into the query side (G = M_k^T @ QhS).

kernel(**inputs) returns (top_idx [512,128] i32, top_scores [512,128] f32,
rerank_scores [512] f32).
"""

import math
import numpy as np

from concourse import bass, bacc, mybir, tile

FP32 = mybir.dt.float32
FP32R = mybir.dt.float32r
BF16 = mybir.dt.bfloat16
U32 = mybir.dt.uint32
I32 = mybir.dt.int32

NEG = -1.0e30
ID = mybir.ActivationFunctionType.Identity
EXP = mybir.ActivationFunctionType.Exp
RELU = mybir.ActivationFunctionType.Relu


def _p(**kw):
    class P:
        pass

    p = P()
    for k, v in kw.items():
        setattr(p, k, v)
    return p


def make_params(B=512, D=256, N=200000, K=128, ncores=8, chunk=1024):
    nshard = N // ncores
    nch = math.ceil(nshard / chunk)
    return _p(
        B=B, D=D, N=N, K=K, ncores=ncores, chunk=chunk,
        nshard=nshard, nch=nch, npad=nch * chunk, qpc=B // ncores,
        cand=nch * 8, mw=ncores * nch * 8,
        nqb=max(B // 128, 1), qb_size=min(B, 128),
        nheads=8, hd=D // 8, dh=D // 2,
    )


def fp32r_round(a):
    """Host-side float32r rounding (round-to-nearest, 11-bit mantissa)."""
    b = np.ascontiguousarray(a, np.float32).view(np.uint32)
    r = (b + np.uint32(0x800)) & np.uint32(0xFFFFF000)
    return r.view(np.float32)


# ---------------------------------------------------------------------------
# Builder
# ---------------------------------------------------------------------------

def build_nc(p):
    nc = bacc.Bacc("TRN2", target_bir_lowering=False, debug=False,
                   num_devices=p.ncores)
    dt = FP32
    KD = 2
    QB, NQB = p.qb_size, p.nqb
    CH, NCH, CAND = p.chunk, p.nch, p.cand
    QPC, MW = p.qpc, p.mw
    ROWW = 2 * CAND

    def din(name, shape, dtype=dt):
        return nc.dram_tensor(name, shape, dtype, kind="ExternalInput")

    qT = din("qT", [128, KD, p.B], FP32R)
    qT_mine = din("qT_mine", [128, KD, QPC])
    eT = din("eT", [128, KD, p.npad], FP32R)
    eFull = din("eFull", [p.N, p.D])
    base_add = din("base_add", [128, CAND], U32)
    qbaseT = din("qbaseT", [128, QPC])       # i*CAND
    qiota = din("qiota", [128, QPC])         # i
    hmask = din("hmask", [128, KD, 8])
    ident = din("ident", [128, 128])
    Wcd = din("Wcd", [128, KD, KD, 128])
    Wk = din("Wk", [128, KD, KD, 128])       # M_k needs untransposed chunks
    WkT = din("WkT", [128, KD, p.D])
    WvT = din("WvT", [128, KD, p.D])
    WcqT = din("WcqT", [128, KD, p.D])
    WqT = din("WqT", [128, KD, p.D])
    WoT = din("WoT", [128, KD, p.D])
    Ws1T = din("Ws1T", [128, KD, p.dh])
    Ws2T = din("Ws2T", [128, 1])
    b_cq = din("b_cq", [128, KD, 1])
    b_cd = din("b_cd", [128, KD, 1])
    b_q = din("b_q", [128, KD, 1])
    b_k = din("b_k", [128, KD, 1])
    b_v = din("b_v", [128, KD, 1])
    b_o = din("b_o", [128, KD, 1])
    b_s1 = din("b_s1", [p.dh, 1])
    b_s2 = din("b_s2", [1, 1])

    out_idx = nc.dram_tensor("top_idx", [QPC, p.K], I32, kind="ExternalOutput")
    out_scores = nc.dram_tensor("top_scores", [QPC, p.K], dt,
                                kind="ExternalOutput")
    out_rr = nc.dram_tensor("rerank", [1, QPC], dt, kind="ExternalOutput")

    a2a_in = nc.dram_tensor("a2a_in", [p.B, ROWW], dt)
    a2a_out = nc.dram_tensor("a2a_out", [p.B, ROWW], dt)
    idx_scratch = nc.dram_tensor("idx_scratch", [p.K * QPC, 64], U32)

    with tile.TileContext(nc) as tc:
        # ================= Stage A: scoring + per-chunk top-8 =============
        with tc.tile_pool(name="qt", bufs=1) as qt_pool, \
             tc.tile_pool(name="et", bufs=3) as et_pool, \
             tc.tile_pool(name="psA", bufs=3, space="PSUM") as psA, \
             tc.tile_pool(name="candp", bufs=1) as cand_pool, \
             tc.tile_pool(name="miscA", bufs=1) as miscA:

            qT_sb = qt_pool.tile([128, KD, p.B], FP32R)
            nc.sync.dma_start(out=qT_sb[:], in_=qT[:])
            base_sb = miscA.tile([128, CAND], U32)
            nc.sync.dma_start(out=base_sb[:], in_=base_add[:])

            packs = [cand_pool.tile([QB, ROWW], U32, tag=f"pack{qb}",
                                    name=f"pack{qb}")
                     for qb in range(NQB)]

            for c in range(NCH):
                et_tile = et_pool.tile([128, KD, CH], FP32R, tag="et")
                nc.sync.dma_start(out=et_tile[:],
                                  in_=eT[:, :, c * CH:(c + 1) * CH])
                for qb in range(NQB):
                    ps = psA.tile([128, CH], FP32, tag="psA")
                    mmw = min(CH, 512)
                    for h in range(CH // mmw):
                        for kc in range(KD):
                            nc.tensor.matmul(
                                out=ps[:QB, h * mmw:(h + 1) * mmw],
                                lhsT=qT_sb[:, kc, qb * QB:(qb + 1) * QB],
                                rhs=et_tile[:, kc, h * mmw:(h + 1) * mmw],
                                start=(kc == 0), stop=(kc == KD - 1))
                    pv = packs[qb][:].rearrange("q (c t) -> q c t", t=2)
                    vals8 = pv[:, c * 8:(c + 1) * 8, 0].bitcast(FP32)
                    nc.vector.max(out=vals8, in_=ps[:QB, :])
                    nc.vector.max_index(
                        out=pv[:, c * 8:(c + 1) * 8, 1],
                        in_max=vals8, in_values=ps[:QB, :])
            for qb in range(NQB):
                pv = packs[qb][:].rearrange("q (c t) -> q c t", t=2)
                nc.vector.tensor_tensor(
                    out=pv[:, :, 1], in0=pv[:, :, 1],
                    in1=base_sb[:QB, :], op=mybir.AluOpType.add)
                nc.sync.dma_start(
                    out=a2a_in[qb * QB:(qb + 1) * QB, :].bitcast(U32),
                    in_=packs[qb][:])

            nc.gpsimd.collective_compute(
                "AllToAll", mybir.AluOpType.bypass,
                replica_groups=[list(range(p.ncores))],
                ins=[a2a_in[:].opt()], outs=[a2a_out[:].opt()])

        # ================= Stage C: global merge ==========================
        with tc.tile_pool(name="mrg", bufs=1) as mrg_pool, \
             tc.tile_pool(name="sel", bufs=1) as sel_pool, \
             tc.tile_pool(name="pst", bufs=1, space="PSUM") as pst:
            # contiguous load of all (val, idx) pairs for my queries;
            # the extraction scans the stride-2 value view in place.
            pairs_sb = mrg_pool.tile([QPC, p.ncores, 2 * CAND], dt)
            nc.sync.dma_start(
                out=pairs_sb[:],
                in_=a2a_out.ap().rearrange("(s i) w -> i s w", s=p.ncores))
            vals_sb = mrg_pool.tile([QPC, MW], dt)
            nc.vector.tensor_copy(
                out=vals_sb[:].rearrange("i (s w) -> i s w", s=p.ncores),
                in_=pairs_sb[:].rearrange("i s (w t) -> i s w t",
                                          t=2)[:, :, :, 0])
            work = vals_sb[:]

            appr_sb = sel_pool.tile([QPC, p.K], dt)
            slots_sb = sel_pool.tile([QPC, p.K], U32)
            nrounds = p.K // 8
            for r in range(nrounds):
                v8 = appr_sb[:, r * 8:(r + 1) * 8]
                nc.vector.max(out=v8, in_=work)
                nc.vector.max_index(out=slots_sb[:, r * 8:(r + 1) * 8],
                                    in_max=v8, in_values=work)
                if r != nrounds - 1:
                    nc.vector.match_replace(out=work, in_to_replace=v8,
                                            in_values=work, imm_value=NEG)

            # ---- slot -> pair index (rank-on-partition orientation) ----
            inv = np.float32(1.0) / np.float32(CAND)
            MAGIC = np.float32(12582912.0)
            FEPS = np.float32(0.4985)
            fl = np.float32(np.float32(np.float32(
                np.arange(MW, dtype=np.float32) * inv) - FEPS) + MAGIC) - MAGIC
            assert np.all(fl == (np.arange(MW) // CAND)), "floor trick invalid"

            id_sb = sel_pool.tile([128, 128], dt)
            nc.sync.dma_start(out=id_sb[:], in_=ident[:])
            qb_sb = sel_pool.tile([128, QPC], dt)
            nc.sync.dma_start(out=qb_sb[:], in_=qbaseT[:])
            sf = sel_pool.tile([QPC, p.K], dt)
            nc.vector.tensor_copy(out=sf[:], in_=slots_sb[:])
            tp = pst.tile([128, 128], FP32, tag="tp")
            nc.tensor.transpose(out=tp[:, :QPC], in_=sf[:],
                                identity=id_sb[:QPC, :QPC])
            sfT = sel_pool.tile([p.K, QPC], dt)
            nc.scalar.activation(out=sfT[:], in_=tp[:p.K, :QPC], func=ID)
            segf = sel_pool.tile([p.K, QPC], dt)
            nc.vector.tensor_scalar(out=segf[:], in0=sfT[:],
                                    scalar1=float(inv), scalar2=float(FEPS),
                                    op0=mybir.AluOpType.mult,
                                    op1=mybir.AluOpType.subtract)
            nc.vector.tensor_scalar(out=segf[:], in0=segf[:],
                                    scalar1=float(MAGIC), scalar2=float(MAGIC),
                                    op0=mybir.AluOpType.add,
                                    op1=mybir.AluOpType.subtract)
            nc.vector.tensor_scalar(out=segf[:], in0=segf[:],
                                    scalar1=float(CAND * (QPC - 1)),
                                    scalar2=None, op0=mybir.AluOpType.mult)
            nc.vector.tensor_add(sfT[:], sfT[:], segf[:])
            nc.vector.tensor_tensor(out=sfT[:], in0=sfT[:],
                                    in1=qb_sb[:p.K, :],
                                    op=mybir.AluOpType.add)
            flat_u = sel_pool.tile([p.K, QPC], U32)
            nc.vector.tensor_copy(out=flat_u[:], in_=sfT[:])

            # ---- per-query gathers: (val, idx) pair then embedding row ----
            a2a_pairs = a2a_out.ap().rearrange("b (w t) -> (b w) t", t=2)
            pairbuf = sel_pool.tile([p.K, QPC, 2], dt)
            cand_all = sel_pool.tile([p.K, QPC, p.D], dt)
            for i in range(QPC):
                nc.gpsimd.indirect_dma_start(
                    out=pairbuf[:, i, :], out_offset=None,
                    in_=a2a_pairs,
                    in_offset=bass.IndirectOffsetOnAxis(
                        ap=flat_u[:, i:i + 1], axis=0))
                nc.gpsimd.indirect_dma_start(
                    out=cand_all[:, i, :], out_offset=None,
                    in_=eFull[:],
                    in_offset=bass.IndirectOffsetOnAxis(
                        ap=pairbuf[:, i, 1:2].bitcast(U32), axis=0))

            # stash gathered ids for the final permutation (col 0 of each
            # 256-byte row so dma_gather's row-size constraint is met)
            nc.sync.dma_start(
                out=idx_scratch[:, 0:1],
                in_=pairbuf[:, :, 1].bitcast(U32).unsqueeze(2))

            _rerank(nc, tc, p, cand_all, pairbuf, idx_scratch, qT_mine,
                    WcqT, WqT, Wcd, Wk, WkT, WvT, WoT, Ws1T, Ws2T, b_cq,
                    b_cd, b_q, b_k, b_v, b_o, b_s1, b_s2, hmask, qiota,
                    id_sb, pst, out_idx, out_scores, out_rr)

    nc.compile()
    return nc


def _rerank(nc, tc, p, cand_all, pairbuf, idx_scratch, qT_mine, WcqT, WqT,
            Wcd, Wk, WkT, WvT, WoT, Ws1T, Ws2T, b_cq, b_cd, b_q, b_k, b_v,
            b_o, b_s1, b_s2, hmask, qiota, id_sb, pst, out_idx, out_scores,
            out_rr):
    dt = FP32
    KD = 2
    QPC = p.qpc

    with tc.tile_pool(name="wts", bufs=1) as wp, \
         tc.tile_pool(name="cT", bufs=6) as cT_pool, \
         tc.tile_pool(name="kvV", bufs=17) as kvV, \
         tc.tile_pool(name="at", bufs=4) as at_pool, \
         tc.tile_pool(name="ctxp", bufs=1) as ctx_pool, \
         tc.tile_pool(name="psB", bufs=3, space="PSUM") as psB, \
         tc.tile_pool(name="psSm", bufs=2, space="PSUM") as psSm:

        def big_ps():
            return psB.tile([128, 512], FP32, tag="big", name="bigps")

        def ldb(t, shape):
            """Load a weight tensor, casting fp32 -> bf16 in the DMA."""
            s = wp.tile(shape, BF16, tag=t.name, name="w_" + t.name)
            nc.gpsimd.dma_start(out=s[:], in_=t[:])
            return s

        def ldf(t, shape):
            s = wp.tile(shape, dt, tag=t.name, name="w_" + t.name)
            nc.sync.dma_start(out=s[:], in_=t[:])
            return s

        wcd = ldb(Wcd, [128, KD, KD, 128])
        wkT = ldb(WkT, [128, KD, p.D])
        wvT = ldb(WvT, [128, KD, p.D])
        wcqT = ldb(WcqT, [128, KD, p.D])
        wqT = ldb(WqT, [128, KD, p.D])
        woT = ldb(WoT, [128, KD, p.D])
        ws1T = ldb(Ws1T, [128, KD, p.dh])
        ws2T = ldb(Ws2T, [128, 1])
        bcq = ldf(b_cq, [128, KD, 1])
        bcd = ldf(b_cd, [128, KD, 1])
        bq = ldf(b_q, [128, KD, 1])
        bk = ldf(b_k, [128, KD, 1])
        bv = ldf(b_v, [128, KD, 1])
        bo = ldf(b_o, [128, KD, 1])
        bs1 = ldf(b_s1, [p.dh, 1])
        bs2 = ldf(b_s2, [1, 1])
        hm = ldb(hmask, [128, KD, 8])
        hmf = ldf(hmask, [128, KD, 8])
        qtm = ldf(qT_mine, [128, KD, QPC])
        qtm_b = wp.tile([128, KD, QPC], BF16, tag="qtmb")
        nc.vector.tensor_copy(out=qtm_b[:], in_=qtm[:])
        qio = ldf(qiota, [128, QPC])

        # M_vT (bf16): [128(j), jc, i] = (Wv @ Wcd)[i, jc*128+j]
        mvT = wp.tile([128, KD, p.D], BF16, tag="mvT")
        for jc in range(KD):
            ps = big_ps()
            for lc in range(KD):
                nc.tensor.matmul(out=ps[:, :p.D], lhsT=wcd[:, lc, jc, :],
                                 rhs=wvT[:, lc, :],
                                 start=(lc == 0), stop=(lc == KD - 1))
            nc.scalar.activation(out=mvT[:, jc, :], in_=ps[:, :p.D], func=ID)
        # M_k untransposed chunks (bf16): mk[128(j), jc(j), ic(j'), 128(j')]
        #   = M_k[jc*128+j, ic*128+j'] = sum_l Wk[j, l] Wcd[l, j']
        mk = wp.tile([128, KD, KD, 128], BF16, tag="mk")
        for jc in range(KD):
            ps = big_ps()
            for lc in range(KD):
                # out[j, j'] = sum_l WkT[l, jc*128+j]^T ... lhsT = wkT chunk
                nc.tensor.matmul(out=ps[:, :p.D],
                                 lhsT=wkT[:, lc, jc * 128:(jc + 1) * 128],
                                 rhs=wcd[:, lc, :, :].rearrange("l a b -> l (a b)"),
                                 start=(lc == 0), stop=(lc == KD - 1))
            nc.scalar.activation(
                out=mk[:, jc, :, :].rearrange("j a b -> j (a b)"),
                in_=ps[:, :p.D], func=ID)

        # c_k = Wk @ b_cd + bk ; c_v = Wv @ b_cd + bv
        bcd_b = wp.tile([128, KD, 1], BF16, tag="bcdb")
        nc.vector.tensor_copy(out=bcd_b[:], in_=bcd[:])
        ck = wp.tile([128, KD, 1], dt, tag="ck")
        cv = wp.tile([128, KD, 1], dt, tag="cv")
        for (ct, wt, bt) in ((ck, wkT, bk), (cv, wvT, bv)):
            for m in range(KD):
                ps = psSm.tile([128, 8], FP32, tag="small", name="smps")
                for jc in range(KD):
                    nc.tensor.matmul(out=ps[:, :1],
                                     lhsT=wt[:, jc, m * 128:(m + 1) * 128],
                                     rhs=bcd_b[:, jc, :],
                                     start=(jc == 0), stop=(jc == KD - 1))
                nc.scalar.activation(out=ct[:, m, :], in_=ps[:, :1], func=ID,
                                     bias=bt[:, m, :])

        # Q'T = W_cq @ qT_mine + b_cq ; qhT = (Wq @ Q'T + bq) / sqrt(hd)
        qpT = wp.tile([128, KD, QPC], BF16, tag="qpT")
        qhT = wp.tile([128, KD, QPC], dt, tag="qhT")
        for m in range(KD):
            ps = big_ps()
            for jc in range(KD):
                nc.tensor.matmul(out=ps[:, :QPC],
                                 lhsT=wcqT[:, jc, m * 128:(m + 1) * 128],
                                 rhs=qtm_b[:, jc, :], start=(jc == 0),
                                 stop=(jc == KD - 1))
            nc.scalar.activation(out=qpT[:, m, :], in_=ps[:, :QPC], func=ID,
                                 bias=bcq[:, m, :])
        sc = 1.0 / math.sqrt(p.hd)
        bq_s = wp.tile([128, KD, 1], dt, tag="bqs")
        nc.vector.tensor_scalar(out=bq_s[:], in0=bq[:], scalar1=sc,
                                scalar2=None, op0=mybir.AluOpType.mult)
        for m in range(KD):
            ps = big_ps()
            for jc in range(KD):
                nc.tensor.matmul(out=ps[:, :QPC],
                                 lhsT=wqT[:, jc, m * 128:(m + 1) * 128],
                                 rhs=qpT[:, jc, :], start=(jc == 0),
                                 stop=(jc == KD - 1))
            nc.scalar.activation(out=qhT[:, m, :], in_=ps[:, :QPC], func=ID,
                                 bias=bq_s[:, m, :], scale=sc)

        # QhS_all[j, jc, q, h] = qhT * hmask ; G = M_k^T @ QhS (bf16)
        qhs_all = wp.tile([128, KD, QPC, 8], BF16, tag="qhs")
        nc.vector.tensor_tensor(
            out=qhs_all[:],
            in0=qhT[:].unsqueeze(3).to_broadcast([128, KD, QPC, 8]),
            in1=hmf[:].unsqueeze(2).to_broadcast([128, KD, QPC, 8]),
            op=mybir.AluOpType.mult)
        g_all = wp.tile([128, KD, QPC, 8], BF16, tag="g_all")
        for jpc in range(KD):
            nmm = (QPC * 8) // 512
            for b in range(max(nmm, 1)):
                w = min(512, QPC * 8)
                ps = big_ps()
                for jc in range(KD):
                    nc.tensor.matmul(
                        out=ps[:, :w],
                        lhsT=mk[:, jc, jpc, :],
                        rhs=qhs_all[:, jc, :, :].rearrange(
                            "j q h -> j (q h)")[:, b * 512:b * 512 + w],
                        start=(jc == 0), stop=(jc == KD - 1))
                nc.scalar.activation(
                    out=g_all[:, jpc, :, :].rearrange(
                        "j q h -> j (q h)")[:, b * 512:b * 512 + w],
                    in_=ps[:, :w], func=ID)

        # ---- per-query: transpose cand, rescore, attention ----
        sexT = ctx_pool.tile([p.K, QPC], dt)
        ctxcol = ctx_pool.tile([128, KD, QPC, 8], dt)
        nbat = max(QPC // 16, 1)
        bsz = QPC // nbat
        for bi in range(nbat):
            attn = at_pool.tile([8, bsz, 128], dt, tag="attn")
            vhs = []
            for qq in range(bsz):
                q = bi * bsz + qq
                candq = cand_all[:, q, :]
                cTf = cT_pool.tile([128, KD, 128], dt, tag="cTf")
                cTb = cT_pool.tile([128, KD, 128], BF16, tag="cTb")
                for m in range(KD):
                    tpp = big_ps()
                    nc.tensor.transpose(out=tpp[:, :128],
                                        in_=candq[:, m * 128:(m + 1) * 128],
                                        identity=id_sb[:])
                    nc.scalar.activation(out=cTf[:, m, :], in_=tpp[:, :128],
                                         func=ID)
                    nc.scalar.activation(out=cTb[:, m, :], in_=tpp[:, :128],
                                         func=ID)
                # exact rescore: s_ex[k] = sum_d candT[d,k] * q[d]
                ps = psSm.tile([128, 8], FP32, tag="small", name="smps2")
                for jc in range(KD):
                    nc.tensor.matmul(out=ps[:, :1], lhsT=cTf[:, jc, :],
                                     rhs=qtm[:, jc, q:q + 1],
                                     start=(jc == 0), stop=(jc == KD - 1))
                nc.scalar.activation(out=sexT[:, q:q + 1], in_=ps[:, :1],
                                     func=ID)
                # vh = cand @ M_v^T + (c_v deferred)  [128k, D] bf16
                vh = kvV.tile([128, p.D], BF16, tag="vh")
                ps2 = big_ps()
                for jc in range(KD):
                    nc.tensor.matmul(out=ps2[:, :p.D], lhsT=cTb[:, jc, :],
                                     rhs=mvT[:, jc, :],
                                     start=(jc == 0), stop=(jc == KD - 1))
                nc.scalar.activation(out=vh[:], in_=ps2[:, :p.D], func=ID)
                vhs.append(vh)
                # logits = G_q^T @ candT
                lg_ps = psSm.tile([8, 128], FP32, tag="lgps", name="lgps")
                for jpc in range(KD):
                    nc.tensor.matmul(out=lg_ps[:],
                                     lhsT=g_all[:, jpc, q, :],
                                     rhs=cTb[:, jpc, :],
                                     start=(jpc == 0), stop=(jpc == KD - 1))
                nc.scalar.activation(out=attn[:, qq, :], in_=lg_ps[:],
                                     func=ID)
            nmax = at_pool.tile([8, bsz], dt, tag="nmax")
            nc.vector.tensor_reduce(out=nmax[:], in_=attn[:],
                                    axis=mybir.AxisListType.X,
                                    op=mybir.AluOpType.max, negate=True)
            nc.vector.tensor_tensor(
                out=attn[:], in0=attn[:],
                in1=nmax[:].unsqueeze(2).to_broadcast([8, bsz, 128]),
                op=mybir.AluOpType.add)
            nc.scalar.activation(out=attn[:], in_=attn[:], func=EXP)
            ssum = at_pool.tile([8, bsz], dt, tag="ssum")
            nc.vector.tensor_reduce(out=ssum[:], in_=attn[:],
                                    axis=mybir.AxisListType.X,
                                    op=mybir.AluOpType.add)
            rinv = at_pool.tile([8, bsz], dt, tag="rinv")
            nc.vector.reciprocal(out=rinv[:], in_=ssum[:])
            nc.vector.tensor_tensor(
                out=attn[:], in0=attn[:],
                in1=rinv[:].unsqueeze(2).to_broadcast([8, bsz, 128]),
                op=mybir.AluOpType.mult)
            for qq in range(bsz):
                q = bi * bsz + qq
                tpp = psSm.tile([128, 8], FP32, tag="small", name="smps3")
                nc.tensor.transpose(out=tpp[:], in_=attn[:, qq, :],
                                    identity=id_sb[:8, :8])
                attnT = at_pool.tile([128, 8], BF16, tag="attnT")
                nc.scalar.activation(out=attnT[:], in_=tpp[:], func=ID)
                for m in range(KD):
                    ps3 = psSm.tile([128, 8], FP32, tag="small", name="smps4")
                    nc.tensor.matmul(out=ps3[:],
                                     lhsT=vhs[qq][:, m * 128:(m + 1) * 128],
                                     rhs=attnT[:], start=True, stop=True)
                    nc.scalar.activation(out=ctxcol[:, m, q, :], in_=ps3[:],
                                         func=ID, bias=cv[:, m, :])

        # ---- exact resort ----
        tp = pst.tile([128, 128], FP32, tag="tp")
        nc.tensor.transpose(out=tp[:QPC, :p.K], in_=sexT[:],
                            identity=id_sb[:])
        sex = ctx_pool.tile([QPC, p.K], dt)
        nc.scalar.activation(out=sex[:], in_=tp[:QPC, :p.K], func=ID)
        swork = ctx_pool.tile([QPC, p.K], dt)
        nc.vector.tensor_copy(out=swork[:], in_=sex[:])
        scr_sb = ctx_pool.tile([QPC, p.K], dt)
        slots3 = ctx_pool.tile([QPC, p.K], U32)
        nr = p.K // 8
        for r in range(nr):
            v8 = scr_sb[:, r * 8:(r + 1) * 8]
            nc.vector.max(out=v8, in_=swork[:])
            nc.vector.max_index(out=slots3[:, r * 8:(r + 1) * 8],
                                in_max=v8, in_values=swork[:])
            if r != nr - 1:
                nc.vector.match_replace(out=swork[:], in_to_replace=v8,
                                        in_values=swork[:], imm_value=NEG)
        nc.sync.dma_start(out=out_scores[:], in_=scr_sb[:])

        # ---- permute ids by exact rank via DRAM scratch ----
        s3f = ctx_pool.tile([QPC, p.K], dt)
        nc.vector.tensor_copy(out=s3f[:], in_=slots3[:])
        tp2 = pst.tile([128, 128], FP32, tag="tp")
        nc.tensor.transpose(out=tp2[:, :QPC], in_=s3f[:],
                            identity=id_sb[:QPC, :QPC])
        f3T = ctx_pool.tile([p.K, QPC], dt)
        # flat = slot*QPC + i
        nc.scalar.activation(out=f3T[:], in_=tp2[:p.K, :QPC], func=ID,
                             scale=float(QPC))
        nc.vector.tensor_tensor(out=f3T[:], in0=f3T[:], in1=qio[:p.K, :],
                                op=mybir.AluOpType.add)
        # batched permutation: one dma_gather over 256-byte scratch rows.
        # idx list order k = q*K + j -> wrapped[(j%16), q*(K//16) + j//16]
        f3i = ctx_pool.tile([p.K, QPC], mybir.dt.int16)
        nc.vector.tensor_copy(out=f3i[:], in_=f3T[:])
        wrapped = ctx_pool.tile([128, QPC * (p.K // 16)], mybir.dt.int16)
        wv = wrapped[:].rearrange("p (q h) -> p q h", h=p.K // 16)
        for hi in range(p.K // 16):
            nc.sync.dma_start(out=wv[:16, :, hi],
                              in_=f3i[hi * 16:(hi + 1) * 16, :])
        for g in range(1, 8):
            nc.sync.dma_start(out=wrapped[g * 16:(g + 1) * 16, :],
                              in_=wrapped[:16, :])
        gbuf = ctx_pool.tile([128, (p.K * QPC) // 128, 64], dt)
        nc.gpsimd.dma_gather(
            out_ap=gbuf[:], in_ap=idx_scratch.ap().bitcast(FP32),
            idxs_ap=wrapped[:],
            num_idxs=p.K * QPC, num_idxs_reg=p.K * QPC, elem_size=64,
            single_packet=False)
        idxff = ctx_pool.tile([p.K, QPC], dt)
        nc.vector.tensor_copy(out=idxff[:], in_=gbuf[:, :, 0].bitcast(U32))
        tp3 = pst.tile([128, 128], FP32, tag="tp")
        nc.tensor.transpose(out=tp3[:QPC, :p.K], in_=idxff[:],
                            identity=id_sb[:])
        idx_out_sb = ctx_pool.tile([QPC, p.K], I32)
        nc.vector.tensor_copy(out=idx_out_sb[:], in_=tp3[:QPC, :p.K])
        nc.sync.dma_start(out=out_idx[:], in_=idx_out_sb[:])


        # ---- diag extract + output head ----
        ctxTf = ctx_pool.tile([128, KD, QPC], dt)
        tmp = ctx_pool.tile([128, KD, QPC, 8], dt)
        nc.vector.tensor_tensor(
            out=tmp[:], in0=ctxcol[:],
            in1=hmf[:].unsqueeze(2).to_broadcast([128, KD, QPC, 8]),
            op=mybir.AluOpType.mult)
        nc.vector.tensor_reduce(out=ctxTf[:], in_=tmp[:],
                                axis=mybir.AxisListType.X,
                                op=mybir.AluOpType.add)
        ctxT = ctx_pool.tile([128, KD, QPC], BF16)
        nc.vector.tensor_copy(out=ctxT[:], in_=ctxTf[:])

        crossT = ctx_pool.tile([128, KD, QPC], BF16)
        for m in range(KD):
            ps = big_ps()
            for jc in range(KD):
                nc.tensor.matmul(out=ps[:, :QPC],
                                 lhsT=woT[:, jc, m * 128:(m + 1) * 128],
                                 rhs=ctxT[:, jc, :], start=(jc == 0),
                                 stop=(jc == KD - 1))
            nc.scalar.activation(out=crossT[:, m, :], in_=ps[:, :QPC],
                                 func=ID, bias=bo[:, m, :])
        s1T = ctx_pool.tile([p.dh, QPC], BF16)
        ps = big_ps()
        for jc in range(KD):
            nc.tensor.matmul(out=ps[:p.dh, :QPC], lhsT=ws1T[:, jc, :],
                             rhs=crossT[:, jc, :], start=(jc == 0),
                             stop=(jc == KD - 1))
        nc.scalar.activation(out=s1T[:], in_=ps[:p.dh, :QPC], func=RELU,
                             bias=bs1[:])
        rr_ps = big_ps()
        nc.tensor.matmul(out=rr_ps[:1, :QPC], lhsT=ws2T[:p.dh, :],
                         rhs=s1T[:], start=True, stop=True)
        rr_sb = ctx_pool.tile([1, QPC], dt)
        nc.scalar.activation(out=rr_sb[:], in_=rr_ps[:1, :QPC], func=ID,
                             bias=bs2[:])
        nc.sync.dma_start(out=out_rr[:], in_=rr_sb[:])


# ---------------------------------------------------------------------------
# Host-side glue
# ---------------------------------------------------------------------------

def _to2chunk(w):
    return np.ascontiguousarray(w.reshape(2, 128, -1).transpose(1, 0, 2))


def prepare_in_maps(p, inputs):
    f32 = np.float32
    q = np.asarray(inputs["query_emb"], f32)
    E = np.asarray(inputs["index_embs"], f32)
    W_cq = np.asarray(inputs["W_cq"], f32)
    b_cq = np.asarray(inputs["b_cq"], f32)
    W_cd = np.asarray(inputs["W_cd"], f32)
    b_cd = np.asarray(inputs["b_cd"], f32)
    ipw = np.asarray(inputs["in_proj_w"], f32)
    ipb = np.asarray(inputs["in_proj_b"], f32)
    Wq, Wk, Wv = ipw[:p.D], ipw[p.D:2 * p.D], ipw[2 * p.D:]
    bq, bk, bv = ipb[:p.D], ipb[p.D:2 * p.D], ipb[2 * p.D:]
    W_o = np.asarray(inputs["out_proj_w"], f32)
    b_o = np.asarray(inputs["out_proj_b"], f32)
    W_s1 = np.asarray(inputs["W_s1"], f32)
    b_s1 = np.asarray(inputs["b_s1"], f32)
    W_s2 = np.asarray(inputs["W_s2"], f32)
    b_s2 = np.asarray(inputs["b_s2"], f32)

    qT = _to2chunk(np.ascontiguousarray(q.T))
    heads = (np.arange(p.D) // p.hd)
    hmask = np.zeros((p.D, 8), f32)
    hmask[np.arange(p.D), heads] = 1.0
    hmask = _to2chunk(hmask)
    ident = np.eye(128, dtype=f32)
    wcd = np.ascontiguousarray(
        W_cd.reshape(2, 128, 2, 128).transpose(1, 0, 2, 3))
    wk = np.ascontiguousarray(
        Wk.reshape(2, 128, 2, 128).transpose(1, 0, 2, 3))

    def T2(w):
        return _to2chunk(np.ascontiguousarray(w.T))

    def bvec(b):
        return np.ascontiguousarray(b.reshape(2, 128, 1).transpose(1, 0, 2))

    common = dict(
        qT=fp32r_round(qT), eFull=E, hmask=hmask, ident=ident,
        Wcd=wcd, Wk=wk, WkT=T2(Wk), WvT=T2(Wv), WcqT=T2(W_cq), WqT=T2(Wq),
        WoT=T2(W_o), Ws1T=T2(W_s1), Ws2T=np.ascontiguousarray(W_s2.T),
        b_cq=bvec(b_cq), b_cd=bvec(b_cd), b_q=bvec(bq), b_k=bvec(bk),
        b_v=bvec(bv), b_o=bvec(b_o),
        b_s1=np.ascontiguousarray(b_s1.reshape(p.dh, 1)),
        b_s2=np.ascontiguousarray(b_s2.reshape(1, 1)),
    )
    in_maps = []
    for c in range(p.ncores):
        esh = E[c * p.nshard:(c + 1) * p.nshard]
        eTc = np.zeros((p.D, p.npad), f32)
        eTc[:, :p.nshard] = esh.T
        base = np.zeros((128, p.cand), np.uint32)
        for ch in range(p.nch):
            base[:, ch * 8:(ch + 1) * 8] = c * p.nshard + ch * p.chunk
        qbase = np.broadcast_to(
            (np.arange(p.qpc, dtype=np.float64) * p.cand).astype(f32),
            (128, p.qpc)).copy()
        qio = np.broadcast_to(
            np.arange(p.qpc, dtype=f32), (128, p.qpc)).copy()
        m = dict(common)
        m["eT"] = fp32r_round(_to2chunk(eTc))
        m["base_add"] = base
        m["qT_mine"] = np.ascontiguousarray(qT[:, :, c * p.qpc:(c + 1) * p.qpc])
        m["qbaseT"] = qbase
        m["qiota"] = qio
        in_maps.append(m)
    return in_maps


_CACHE = {}


def _get_nc(p):
    key = (p.B, p.N, p.chunk)
    if key not in _CACHE:
        _CACHE[key] = build_nc(p)
    return _CACHE[key]


def run(inputs, trace=False, **kw):
    from concourse.bass_utils import run_bass_kernel_spmd
    p = make_params()
    nc = _get_nc(p)
    in_maps = prepare_in_maps(p, inputs)
    res = run_bass_kernel_spmd(nc, in_maps, core_ids=list(range(p.ncores)),
                               trace=trace, **kw)
    outs = res.results
    top_idx = np.concatenate([o["top_idx"] for o in outs], axis=0)
    top_scores = np.concatenate([o["top_scores"] for o in outs], axis=0)
    rerank = np.concatenate([o["rerank"].reshape(-1) for o in outs], axis=0)
    return (top_idx.astype(np.int32), top_scores, rerank), res


def kernel(**inputs):
    out, _ = run(inputs, trace=False)
    return out


# revision 47
# speedup vs baseline: 1.1553x; 1.0065x over previous
"""Trainium2 distributed kernel for nn_AdMatcher (retrieval_knn).

Strategy (8 NeuronCores, SPMD):
  - index_embs sharded row-wise; scoring matmul in float32r (TF32-grade,
    1 cyc/col) -> chunked per-1024-col top-8 via DVE max/max_index.
  - AllToAll exchanges (val, idx) candidate pairs; each core extracts the
    approx global top-128 for its 64 queries (iterative max8/match_replace).
  - Winner (val, idx) pairs and embedding rows gathered via SWDGE indirect
    DMA (one offset per partition); exact fp32 rescore on PE + resort gives
    the exact top-128 order/values; idx permuted through a DRAM scratch.
  - Rerank cross-attention + MLP runs in bf16, query-parallel, with the
    kh projection folded into the query side (G = M_k^T @ QhS).

kernel(**inputs) returns (top_idx [512,128] i32, top_scores [512,128] f32,
rerank_scores [512] f32).
"""

import math
import numpy as np

from concourse import bass, bacc, mybir, tile

FP32 = mybir.dt.float32
FP32R = mybir.dt.float32r
BF16 = mybir.dt.bfloat16
U32 = mybir.dt.uint32
I32 = mybir.dt.int32

NEG = -1.0e30
ID = mybir.ActivationFunctionType.Identity
EXP = mybir.ActivationFunctionType.Exp
RELU = mybir.ActivationFunctionType.Relu


def _p(**kw):
    class P:
        pass

    p = P()
    for k, v in kw.items():
        setattr(p, k, v)
    return p


def make_params(B=512, D=256, N=200000, K=128, ncores=8, chunk=1024):
    nshard = N // ncores
    nch = math.ceil(nshard / chunk)
    return _p(
        B=B, D=D, N=N, K=K, ncores=ncores, chunk=chunk,
        nshard=nshard, nch=nch, npad=nch * chunk, qpc=B // ncores,
        cand=nch * 8, mw=ncores * nch * 8,
        nqb=max(B // 128, 1), qb_size=min(B, 128),
        nheads=8, hd=D // 8, dh=D // 2,
    )


def fp32r_round(a):
    """Host-side float32r rounding (round-to-nearest, 11-bit mantissa)."""
    b = np.ascontiguousarray(a, np.float32).view(np.uint32)
    r = (b + np.uint32(0x800)) & np.uint32(0xFFFFF000)
    return r.view(np.float32)


# ---------------------------------------------------------------------------
# Builder
# ---------------------------------------------------------------------------

def build_nc(p):
    nc = bacc.Bacc("TRN2", target_bir_lowering=False, debug=False,
                   num_devices=p.ncores)
    dt = FP32
    KD = 2
    QB, NQB = p.qb_size, p.nqb
    CH, NCH, CAND = p.chunk, p.nch, p.cand
    QPC, MW = p.qpc, p.mw
    ROWW = 2 * CAND

    def din(name, shape, dtype=dt):
        return nc.dram_tensor(name, shape, dtype, kind="ExternalInput")

    qT = din("qT", [128, KD, p.B], FP32R)
    qT_mine = din("qT_mine", [128, KD, QPC])
    eT = din("eT", [128, KD, p.npad], FP32R)
    eFull = din("eFull", [p.N, p.D])
    base_add = din("base_add", [128, CAND], U32)
    qbaseT = din("qbaseT", [128, QPC])       # i*CAND
    qiota = din("qiota", [128, QPC])         # i
    hmask = din("hmask", [128, KD, 8])
    ident = din("ident", [128, 128])
    Wcd = din("Wcd", [128, KD, KD, 128])
    Wk = din("Wk", [128, KD, KD, 128])       # M_k needs untransposed chunks
    WkT = din("WkT", [128, KD, p.D])
    WvT = din("WvT", [128, KD, p.D])
    WcqT = din("WcqT", [128, KD, p.D])
    WqT = din("WqT", [128, KD, p.D])
    WoT = din("WoT", [128, KD, p.D])
    Ws1T = din("Ws1T", [128, KD, p.dh])
    Ws2T = din("Ws2T", [128, 1])
    b_cq = din("b_cq", [128, KD, 1])
    b_cd = din("b_cd", [128, KD, 1])
    b_q = din("b_q", [128, KD, 1])
    b_k = din("b_k", [128, KD, 1])
    b_v = din("b_v", [128, KD, 1])
    b_o = din("b_o", [128, KD, 1])
    b_s1 = din("b_s1", [p.dh, 1])
    b_s2 = din("b_s2", [1, 1])

    out_idx = nc.dram_tensor("top_idx", [QPC, p.K], I32, kind="ExternalOutput")
    out_scores = nc.dram_tensor("top_scores", [QPC, p.K], dt,
                                kind="ExternalOutput")
    out_rr = nc.dram_tensor("rerank", [1, QPC], dt, kind="ExternalOutput")

    a2a_in = nc.dram_tensor("a2a_in", [p.B, ROWW], dt)
    a2a_out = nc.dram_tensor("a2a_out", [p.B, ROWW], dt)
    idx_scratch = nc.dram_tensor("idx_scratch", [p.K * QPC, 64], U32)

    with tile.TileContext(nc) as tc:
        # ================= Stage A: scoring + per-chunk top-8 =============
        with tc.tile_pool(name="qt", bufs=1) as qt_pool, \
             tc.tile_pool(name="et", bufs=3) as et_pool, \
             tc.tile_pool(name="psA", bufs=4, space="PSUM") as psA, \
             tc.tile_pool(name="candp", bufs=1) as cand_pool, \
             tc.tile_pool(name="miscA", bufs=1) as miscA:

            qT_sb = qt_pool.tile([128, KD, p.B], FP32R)
            nc.sync.dma_start(out=qT_sb[:], in_=qT[:])
            base_sb = miscA.tile([128, CAND], U32)
            nc.sync.dma_start(out=base_sb[:], in_=base_add[:])

            packs = [cand_pool.tile([QB, ROWW], U32, tag=f"pack{qb}",
                                    name=f"pack{qb}")
                     for qb in range(NQB)]

            for c in range(NCH):
                et_tile = et_pool.tile([128, KD, CH], FP32R, tag="et")
                nc.sync.dma_start(out=et_tile[:],
                                  in_=eT[:, :, c * CH:(c + 1) * CH])
                for qb in range(NQB):
                    ps = psA.tile([128, CH], FP32, tag="psA")
                    mmw = min(CH, 512)
                    for h in range(CH // mmw):
                        for kc in range(KD):
                            nc.tensor.matmul(
                                out=ps[:QB, h * mmw:(h + 1) * mmw],
                                lhsT=qT_sb[:, kc, qb * QB:(qb + 1) * QB],
                                rhs=et_tile[:, kc, h * mmw:(h + 1) * mmw],
                                start=(kc == 0), stop=(kc == KD - 1))
                    pv = packs[qb][:].rearrange("q (c t) -> q c t", t=2)
                    vals8 = pv[:, c * 8:(c + 1) * 8, 0].bitcast(FP32)
                    nc.vector.max(out=vals8, in_=ps[:QB, :])
                    nc.vector.max_index(
                        out=pv[:, c * 8:(c + 1) * 8, 1],
                        in_max=vals8, in_values=ps[:QB, :])
            for qb in range(NQB):
                pv = packs[qb][:].rearrange("q (c t) -> q c t", t=2)
                nc.vector.tensor_tensor(
                    out=pv[:, :, 1], in0=pv[:, :, 1],
                    in1=base_sb[:QB, :], op=mybir.AluOpType.add)
                nc.sync.dma_start(
                    out=a2a_in[qb * QB:(qb + 1) * QB, :].bitcast(U32),
                    in_=packs[qb][:])

            nc.gpsimd.collective_compute(
                "AllToAll", mybir.AluOpType.bypass,
                replica_groups=[list(range(p.ncores))],
                ins=[a2a_in[:].opt()], outs=[a2a_out[:].opt()])

        # ================= Stage C: global merge ==========================
        with tc.tile_pool(name="mrg", bufs=1) as mrg_pool, \
             tc.tile_pool(name="sel", bufs=1) as sel_pool, \
             tc.tile_pool(name="pst", bufs=1, space="PSUM") as pst:
            # contiguous load of all (val, idx) pairs for my queries;
            # the extraction scans the stride-2 value view in place.
            pairs_sb = mrg_pool.tile([QPC, p.ncores, 2 * CAND], dt)
            nc.sync.dma_start(
                out=pairs_sb[:],
                in_=a2a_out.ap().rearrange("(s i) w -> i s w", s=p.ncores))
            vals_sb = mrg_pool.tile([QPC, MW], dt)
            nc.vector.tensor_copy(
                out=vals_sb[:].rearrange("i (s w) -> i s w", s=p.ncores),
                in_=pairs_sb[:].rearrange("i s (w t) -> i s w t",
                                          t=2)[:, :, :, 0])
            work = vals_sb[:]

            appr_sb = sel_pool.tile([QPC, p.K], dt)
            slots_sb = sel_pool.tile([QPC, p.K], U32)
            nrounds = p.K // 8
            for r in range(nrounds):
                v8 = appr_sb[:, r * 8:(r + 1) * 8]
                nc.vector.max(out=v8, in_=work)
                nc.vector.max_index(out=slots_sb[:, r * 8:(r + 1) * 8],
                                    in_max=v8, in_values=work)
                if r != nrounds - 1:
                    nc.vector.match_replace(out=work, in_to_replace=v8,
                                            in_values=work, imm_value=NEG)

            # ---- slot -> pair index (rank-on-partition orientation) ----
            inv = np.float32(1.0) / np.float32(CAND)
            MAGIC = np.float32(12582912.0)
            FEPS = np.float32(0.4985)
            fl = np.float32(np.float32(np.float32(
                np.arange(MW, dtype=np.float32) * inv) - FEPS) + MAGIC) - MAGIC
            assert np.all(fl == (np.arange(MW) // CAND)), "floor trick invalid"

            id_sb = sel_pool.tile([128, 128], dt)
            nc.sync.dma_start(out=id_sb[:], in_=ident[:])
            qb_sb = sel_pool.tile([128, QPC], dt)
            nc.sync.dma_start(out=qb_sb[:], in_=qbaseT[:])
            sf = sel_pool.tile([QPC, p.K], dt)
            nc.vector.tensor_copy(out=sf[:], in_=slots_sb[:])
            tp = pst.tile([128, 128], FP32, tag="tp")
            nc.tensor.transpose(out=tp[:, :QPC], in_=sf[:],
                                identity=id_sb[:QPC, :QPC])
            sfT = sel_pool.tile([p.K, QPC], dt)
            nc.scalar.activation(out=sfT[:], in_=tp[:p.K, :QPC], func=ID)
            segf = sel_pool.tile([p.K, QPC], dt)
            nc.vector.tensor_scalar(out=segf[:], in0=sfT[:],
                                    scalar1=float(inv), scalar2=float(FEPS),
                                    op0=mybir.AluOpType.mult,
                                    op1=mybir.AluOpType.subtract)
            nc.vector.tensor_scalar(out=segf[:], in0=segf[:],
                                    scalar1=float(MAGIC), scalar2=float(MAGIC),
                                    op0=mybir.AluOpType.add,
                                    op1=mybir.AluOpType.subtract)
            nc.vector.tensor_scalar(out=segf[:], in0=segf[:],
                                    scalar1=float(CAND * (QPC - 1)),
                                    scalar2=None, op0=mybir.AluOpType.mult)
            nc.vector.tensor_add(sfT[:], sfT[:], segf[:])
            nc.vector.tensor_tensor(out=sfT[:], in0=sfT[:],
                                    in1=qb_sb[:p.K, :],
                                    op=mybir.AluOpType.add)
            flat_u = sel_pool.tile([p.K, QPC], U32)
            nc.vector.tensor_copy(out=flat_u[:], in_=sfT[:])

            # ---- per-query gathers: (val, idx) pair then embedding row ----
            a2a_pairs = a2a_out.ap().rearrange("b (w t) -> (b w) t", t=2)
            pairbuf = sel_pool.tile([p.K, QPC, 2], dt)
            cand_all = sel_pool.tile([p.K, QPC, p.D], dt)
            for i in range(QPC):
                nc.gpsimd.indirect_dma_start(
                    out=pairbuf[:, i, :], out_offset=None,
                    in_=a2a_pairs,
                    in_offset=bass.IndirectOffsetOnAxis(
                        ap=flat_u[:, i:i + 1], axis=0))
                nc.gpsimd.indirect_dma_start(
                    out=cand_all[:, i, :], out_offset=None,
                    in_=eFull[:],
                    in_offset=bass.IndirectOffsetOnAxis(
                        ap=pairbuf[:, i, 1:2].bitcast(U32), axis=0))

            # stash gathered ids for the final permutation (col 0 of each
            # 256-byte row so dma_gather's row-size constraint is met)
            nc.sync.dma_start(
                out=idx_scratch[:, 0:1],
                in_=pairbuf[:, :, 1].bitcast(U32).unsqueeze(2))

            _rerank(nc, tc, p, cand_all, pairbuf, idx_scratch, qT_mine,
                    WcqT, WqT, Wcd, Wk, WkT, WvT, WoT, Ws1T, Ws2T, b_cq,
                    b_cd, b_q, b_k, b_v, b_o, b_s1, b_s2, hmask, qiota,
                    id_sb, pst, out_idx, out_scores, out_rr)

    nc.compile()
    return nc


def _rerank(nc, tc, p, cand_all, pairbuf, idx_scratch, qT_mine, WcqT, WqT,
            Wcd, Wk, WkT, WvT, WoT, Ws1T, Ws2T, b_cq, b_cd, b_q, b_k, b_v,
            b_o, b_s1, b_s2, hmask, qiota, id_sb, pst, out_idx, out_scores,
            out_rr):
    dt = FP32
    KD = 2
    QPC = p.qpc

    with tc.tile_pool(name="wts", bufs=1) as wp, \
         tc.tile_pool(name="cT", bufs=6) as cT_pool, \
         tc.tile_pool(name="kvV", bufs=17) as kvV, \
         tc.tile_pool(name="at", bufs=4) as at_pool, \
         tc.tile_pool(name="ctxp", bufs=1) as ctx_pool, \
         tc.tile_pool(name="psB", bufs=3, space="PSUM") as psB, \
         tc.tile_pool(name="psSm", bufs=2, space="PSUM") as psSm:

        def big_ps():
            return psB.tile([128, 512], FP32, tag="big", name="bigps")

        def ldb(t, shape):
            """Load a weight tensor, casting fp32 -> bf16 in the DMA."""
            s = wp.tile(shape, BF16, tag=t.name, name="w_" + t.name)
            nc.gpsimd.dma_start(out=s[:], in_=t[:])
            return s

        def ldf(t, shape):
            s = wp.tile(shape, dt, tag=t.name, name="w_" + t.name)
            nc.sync.dma_start(out=s[:], in_=t[:])
            return s

        wcd = ldb(Wcd, [128, KD, KD, 128])
        wkT = ldb(WkT, [128, KD, p.D])
        wvT = ldb(WvT, [128, KD, p.D])
        wcqT = ldb(WcqT, [128, KD, p.D])
        wqT = ldb(WqT, [128, KD, p.D])
        woT = ldb(WoT, [128, KD, p.D])
        ws1T = ldb(Ws1T, [128, KD, p.dh])
        ws2T = ldb(Ws2T, [128, 1])
        bcq = ldf(b_cq, [128, KD, 1])
        bcd = ldf(b_cd, [128, KD, 1])
        bq = ldf(b_q, [128, KD, 1])
        bk = ldf(b_k, [128, KD, 1])
        bv = ldf(b_v, [128, KD, 1])
        bo = ldf(b_o, [128, KD, 1])
        bs1 = ldf(b_s1, [p.dh, 1])
        bs2 = ldf(b_s2, [1, 1])
        hm = ldb(hmask, [128, KD, 8])
        hmf = ldf(hmask, [128, KD, 8])
        qtm = ldf(qT_mine, [128, KD, QPC])
        qtm_b = wp.tile([128, KD, QPC], BF16, tag="qtmb")
        nc.vector.tensor_copy(out=qtm_b[:], in_=qtm[:])
        qio = ldf(qiota, [128, QPC])

        # M_vT (bf16): [128(j), jc, i] = (Wv @ Wcd)[i, jc*128+j]
        mvT = wp.tile([128, KD, p.D], BF16, tag="mvT")
        for jc in range(KD):
            ps = big_ps()
            for lc in range(KD):
                nc.tensor.matmul(out=ps[:, :p.D], lhsT=wcd[:, lc, jc, :],
                                 rhs=wvT[:, lc, :],
                                 start=(lc == 0), stop=(lc == KD - 1))
            nc.scalar.activation(out=mvT[:, jc, :], in_=ps[:, :p.D], func=ID)
        # M_k untransposed chunks (bf16): mk[128(j), jc(j), ic(j'), 128(j')]
        #   = M_k[jc*128+j, ic*128+j'] = sum_l Wk[j, l] Wcd[l, j']
        mk = wp.tile([128, KD, KD, 128], BF16, tag="mk")
        for jc in range(KD):
            ps = big_ps()
            for lc in range(KD):
                # out[j, j'] = sum_l WkT[l, jc*128+j]^T ... lhsT = wkT chunk
                nc.tensor.matmul(out=ps[:, :p.D],
                                 lhsT=wkT[:, lc, jc * 128:(jc + 1) * 128],
                                 rhs=wcd[:, lc, :, :].rearrange("l a b -> l (a b)"),
                                 start=(lc == 0), stop=(lc == KD - 1))
            nc.scalar.activation(
                out=mk[:, jc, :, :].rearrange("j a b -> j (a b)"),
                in_=ps[:, :p.D], func=ID)

        # c_k = Wk @ b_cd + bk ; c_v = Wv @ b_cd + bv
        bcd_b = wp.tile([128, KD, 1], BF16, tag="bcdb")
        nc.vector.tensor_copy(out=bcd_b[:], in_=bcd[:])
        ck = wp.tile([128, KD, 1], dt, tag="ck")
        cv = wp.tile([128, KD, 1], dt, tag="cv")
        for (ct, wt, bt) in ((ck, wkT, bk), (cv, wvT, bv)):
            for m in range(KD):
                ps = psSm.tile([128, 8], FP32, tag="small", name="smps")
                for jc in range(KD):
                    nc.tensor.matmul(out=ps[:, :1],
                                     lhsT=wt[:, jc, m * 128:(m + 1) * 128],
                                     rhs=bcd_b[:, jc, :],
                                     start=(jc == 0), stop=(jc == KD - 1))
                nc.scalar.activation(out=ct[:, m, :], in_=ps[:, :1], func=ID,
                                     bias=bt[:, m, :])

        # Q'T = W_cq @ qT_mine + b_cq ; qhT = (Wq @ Q'T + bq) / sqrt(hd)
        qpT = wp.tile([128, KD, QPC], BF16, tag="qpT")
        qhT = wp.tile([128, KD, QPC], dt, tag="qhT")
        for m in range(KD):
            ps = big_ps()
            for jc in range(KD):
                nc.tensor.matmul(out=ps[:, :QPC],
                                 lhsT=wcqT[:, jc, m * 128:(m + 1) * 128],
                                 rhs=qtm_b[:, jc, :], start=(jc == 0),
                                 stop=(jc == KD - 1))
            nc.scalar.activation(out=qpT[:, m, :], in_=ps[:, :QPC], func=ID,
                                 bias=bcq[:, m, :])
        sc = 1.0 / math.sqrt(p.hd)
        bq_s = wp.tile([128, KD, 1], dt, tag="bqs")
        nc.vector.tensor_scalar(out=bq_s[:], in0=bq[:], scalar1=sc,
                                scalar2=None, op0=mybir.AluOpType.mult)
        for m in range(KD):
            ps = big_ps()
            for jc in range(KD):
                nc.tensor.matmul(out=ps[:, :QPC],
                                 lhsT=wqT[:, jc, m * 128:(m + 1) * 128],
                                 rhs=qpT[:, jc, :], start=(jc == 0),
                                 stop=(jc == KD - 1))
            nc.scalar.activation(out=qhT[:, m, :], in_=ps[:, :QPC], func=ID,
                                 bias=bq_s[:, m, :], scale=sc)

        # QhS_all[j, jc, q, h] = qhT * hmask ; G = M_k^T @ QhS (bf16)
        qhs_all = wp.tile([128, KD, QPC, 8], BF16, tag="qhs")
        nc.vector.tensor_tensor(
            out=qhs_all[:],
            in0=qhT[:].unsqueeze(3).to_broadcast([128, KD, QPC, 8]),
            in1=hmf[:].unsqueeze(2).to_broadcast([128, KD, QPC, 8]),
            op=mybir.AluOpType.mult)
        g_all = wp.tile([128, KD, QPC, 8], BF16, tag="g_all")
        for jpc in range(KD):
            nmm = (QPC * 8) // 512
            for b in range(max(nmm, 1)):
                w = min(512, QPC * 8)
                ps = big_ps()
                for jc in range(KD):
                    nc.tensor.matmul(
                        out=ps[:, :w],
                        lhsT=mk[:, jc, jpc, :],
                        rhs=qhs_all[:, jc, :, :].rearrange(
                            "j q h -> j (q h)")[:, b * 512:b * 512 + w],
                        start=(jc == 0), stop=(jc == KD - 1))
                nc.scalar.activation(
                    out=g_all[:, jpc, :, :].rearrange(
                        "j q h -> j (q h)")[:, b * 512:b * 512 + w],
                    in_=ps[:, :w], func=ID)

        # ---- per-query: transpose cand, rescore, attention ----
        sexT = ctx_pool.tile([p.K, QPC], dt)
        ctxcol = ctx_pool.tile([128, KD, QPC, 8], dt)
        nbat = max(QPC // 16, 1)
        bsz = QPC // nbat
        for bi in range(nbat):
            attn = at_pool.tile([8, bsz, 128], dt, tag="attn")
            vhs = []
            for qq in range(bsz):
                q = bi * bsz + qq
                candq = cand_all[:, q, :]
                cTf = cT_pool.tile([128, KD, 128], dt, tag="cTf")
                cTb = cT_pool.tile([128, KD, 128], BF16, tag="cTb")
                for m in range(KD):
                    tpp = big_ps()
                    nc.tensor.transpose(out=tpp[:, :128],
                                        in_=candq[:, m * 128:(m + 1) * 128],
                                        identity=id_sb[:])
                    nc.scalar.activation(out=cTf[:, m, :], in_=tpp[:, :128],
                                         func=ID)
                    nc.scalar.activation(out=cTb[:, m, :], in_=tpp[:, :128],
                                         func=ID)
                # exact rescore: s_ex[k] = sum_d candT[d,k] * q[d]
                ps = psSm.tile([128, 8], FP32, tag="small", name="smps2")
                for jc in range(KD):
                    nc.tensor.matmul(out=ps[:, :1], lhsT=cTf[:, jc, :],
                                     rhs=qtm[:, jc, q:q + 1],
                                     start=(jc == 0), stop=(jc == KD - 1))
                nc.scalar.activation(out=sexT[:, q:q + 1], in_=ps[:, :1],
                                     func=ID)
                # vh = cand @ M_v^T + (c_v deferred)  [128k, D] bf16
                vh = kvV.tile([128, p.D], BF16, tag="vh")
                ps2 = big_ps()
                for jc in range(KD):
                    nc.tensor.matmul(out=ps2[:, :p.D], lhsT=cTb[:, jc, :],
                                     rhs=mvT[:, jc, :],
                                     start=(jc == 0), stop=(jc == KD - 1))
                nc.scalar.activation(out=vh[:], in_=ps2[:, :p.D], func=ID)
                vhs.append(vh)
                # logits = G_q^T @ candT
                lg_ps = psSm.tile([8, 128], FP32, tag="lgps", name="lgps")
                for jpc in range(KD):
                    nc.tensor.matmul(out=lg_ps[:],
                                     lhsT=g_all[:, jpc, q, :],
                                     rhs=cTb[:, jpc, :],
                                     start=(jpc == 0), stop=(jpc == KD - 1))
                nc.scalar.activation(out=attn[:, qq, :], in_=lg_ps[:],
                                     func=ID)
            nmax = at_pool.tile([8, bsz], dt, tag="nmax")
            nc.vector.tensor_reduce(out=nmax[:], in_=attn[:],
                                    axis=mybir.AxisListType.X,
                                    op=mybir.AluOpType.max, negate=True)
            nc.vector.tensor_tensor(
                out=attn[:], in0=attn[:],
                in1=nmax[:].unsqueeze(2).to_broadcast([8, bsz, 128]),
                op=mybir.AluOpType.add)
            nc.scalar.activation(out=attn[:], in_=attn[:], func=EXP)
            ssum = at_pool.tile([8, bsz], dt, tag="ssum")
            nc.vector.tensor_reduce(out=ssum[:], in_=attn[:],
                                    axis=mybir.AxisListType.X,
                                    op=mybir.AluOpType.add)
            rinv = at_pool.tile([8, bsz], dt, tag="rinv")
            nc.vector.reciprocal(out=rinv[:], in_=ssum[:])
            nc.vector.tensor_tensor(
                out=attn[:], in0=attn[:],
                in1=rinv[:].unsqueeze(2).to_broadcast([8, bsz, 128]),
                op=mybir.AluOpType.mult)
            for qq in range(bsz):
                q = bi * bsz + qq
                tpp = psSm.tile([128, 8], FP32, tag="small", name="smps3")
                nc.tensor.transpose(out=tpp[:], in_=attn[:, qq, :],
                                    identity=id_sb[:8, :8])
                attnT = at_pool.tile([128, 8], BF16, tag="attnT")
                nc.scalar.activation(out=attnT[:], in_=tpp[:], func=ID)
                for m in range(KD):
                    ps3 = psSm.tile([128, 8], FP32, tag="small", name="smps4")
                    nc.tensor.matmul(out=ps3[:],
                                     lhsT=vhs[qq][:, m * 128:(m + 1) * 128],
                                     rhs=attnT[:], start=True, stop=True)
                    nc.scalar.activation(out=ctxcol[:, m, q, :], in_=ps3[:],
                                         func=ID, bias=cv[:, m, :])

        # ---- exact resort ----
        tp = pst.tile([128, 128], FP32, tag="tp")
        nc.tensor.transpose(out=tp[:QPC, :p.K], in_=sexT[:],
                            identity=id_sb[:])
        sex = ctx_pool.tile([QPC, p.K], dt)
        nc.scalar.activation(out=sex[:], in_=tp[:QPC, :p.K], func=ID)
        swork = ctx_pool.tile([QPC, p.K], dt)
        nc.vector.tensor_copy(out=swork[:], in_=sex[:])
        scr_sb = ctx_pool.tile([QPC, p.K], dt)
        slots3 = ctx_pool.tile([QPC, p.K], U32)
        nr = p.K // 8
        for r in range(nr):
            v8 = scr_sb[:, r * 8:(r + 1) * 8]
            nc.vector.max(out=v8, in_=swork[:])
            nc.vector.max_index(out=slots3[:, r * 8:(r + 1) * 8],
                                in_max=v8, in_values=swork[:])
            if r != nr - 1:
                nc.vector.match_replace(out=swork[:], in_to_replace=v8,
                                        in_values=swork[:], imm_value=NEG)
        nc.sync.dma_start(out=out_scores[:], in_=scr_sb[:])

        # ---- permute ids by exact rank via DRAM scratch ----
        # pipelined by rank halves: the dma_gather for ranks [0,64) runs
        # while the resort rounds for ranks [64,128) still execute.
        for hh in range(2):
            j0 = hh * (p.K // 2)
            s3f = ctx_pool.tile([QPC, p.K // 2], dt, tag="s3f",
                                name=f"s3f{hh}")
            nc.vector.tensor_copy(out=s3f[:],
                                  in_=slots3[:, j0:j0 + p.K // 2])
            tp2 = pst.tile([128, 128], FP32, tag="tp", name=f"tp2{hh}")
            nc.tensor.transpose(out=tp2[:p.K // 2, :QPC], in_=s3f[:],
                                identity=id_sb[:QPC, :QPC])
            f3T = ctx_pool.tile([p.K // 2, QPC], dt, tag="f3T",
                                name=f"f3T{hh}")
            nc.scalar.activation(out=f3T[:], in_=tp2[:p.K // 2, :QPC],
                                 func=ID, scale=float(QPC))
            nc.vector.tensor_tensor(out=f3T[:], in0=f3T[:],
                                    in1=qio[:p.K // 2, :],
                                    op=mybir.AluOpType.add)
            f3i = ctx_pool.tile([p.K // 2, QPC], mybir.dt.int16, tag="f3i",
                                name=f"f3i{hh}")
            nc.vector.tensor_copy(out=f3i[:], in_=f3T[:])
            # idx list order k = q*(K/2) + j -> wrapped[(j%16), q*(K/32)+j//16]
            nh = p.K // 32          # hi values per half (4 for K=128)
            wrapped = ctx_pool.tile([128, QPC * nh], mybir.dt.int16,
                                    tag="wrap", name=f"wrap{hh}")
            wv = wrapped[:].rearrange("pp (q h) -> pp q h", h=nh)
            for hi in range(nh):
                nc.sync.dma_start(out=wv[:16, :, hi],
                                  in_=f3i[hi * 16:(hi + 1) * 16, :])
            for g in range(1, 8):
                nc.sync.dma_start(out=wrapped[g * 16:(g + 1) * 16, :],
                                  in_=wrapped[:16, :])
            nidx = (p.K // 2) * QPC
            gbuf = ctx_pool.tile([128, nidx // 128, 64], dt, tag="gbuf",
                                 name=f"gbuf{hh}")
            nc.gpsimd.dma_gather(
                out_ap=gbuf[:], in_ap=idx_scratch.ap().bitcast(FP32),
                idxs_ap=wrapped[:],
                num_idxs=nidx, num_idxs_reg=nidx, elem_size=64,
                single_packet=False)
            # out[p, b] = idx(q = 2b + p//64, j = p%64): transpose, then
            # rows b hold [q=2b ranks | q=2b+1 ranks]
            gf = ctx_pool.tile([128, nidx // 128], dt, tag="gf",
                               name=f"gf{hh}")
            nc.vector.tensor_copy(out=gf[:], in_=gbuf[:, :, 0].bitcast(U32))
            tp4 = pst.tile([128, 128], FP32, tag="tp", name=f"tp4{hh}")
            nc.tensor.transpose(out=tp4[:nidx // 128, :128], in_=gf[:],
                                identity=id_sb[:])
            tsb = ctx_pool.tile([nidx // 128, 128], I32, tag="tsb",
                                name=f"tsb{hh}")
            nc.vector.tensor_copy(out=tsb[:], in_=tp4[:nidx // 128, :128])
            nc.sync.dma_start(
                out=out_idx.ap()[:, j0:j0 + p.K // 2].rearrange(
                    "(b par) j -> b par j", par=2),
                in_=tsb[:].rearrange("b (par j) -> b par j", par=2))

        # ---- diag extract + output head ----
        ctxTf = ctx_pool.tile([128, KD, QPC], dt)
        tmp = ctx_pool.tile([128, KD, QPC, 8], dt)
        nc.vector.tensor_tensor(
            out=tmp[:], in0=ctxcol[:],
            in1=hmf[:].unsqueeze(2).to_broadcast([128, KD, QPC, 8]),
            op=mybir.AluOpType.mult)
        nc.vector.tensor_reduce(out=ctxTf[:], in_=tmp[:],
                                axis=mybir.AxisListType.X,
                                op=mybir.AluOpType.add)
        ctxT = ctx_pool.tile([128, KD, QPC], BF16)
        nc.vector.tensor_copy(out=ctxT[:], in_=ctxTf[:])

        crossT = ctx_pool.tile([128, KD, QPC], BF16)
        for m in range(KD):
            ps = big_ps()
            for jc in range(KD):
                nc.tensor.matmul(out=ps[:, :QPC],
                                 lhsT=woT[:, jc, m * 128:(m + 1) * 128],
                                 rhs=ctxT[:, jc, :], start=(jc == 0),
                                 stop=(jc == KD - 1))
            nc.scalar.activation(out=crossT[:, m, :], in_=ps[:, :QPC],
                                 func=ID, bias=bo[:, m, :])
        s1T = ctx_pool.tile([p.dh, QPC], BF16)
        ps = big_ps()
        for jc in range(KD):
            nc.tensor.matmul(out=ps[:p.dh, :QPC], lhsT=ws1T[:, jc, :],
                             rhs=crossT[:, jc, :], start=(jc == 0),
                             stop=(jc == KD - 1))
        nc.scalar.activation(out=s1T[:], in_=ps[:p.dh, :QPC], func=RELU,
                             bias=bs1[:])
        rr_ps = big_ps()
        nc.tensor.matmul(out=rr_ps[:1, :QPC], lhsT=ws2T[:p.dh, :],
                         rhs=s1T[:], start=True, stop=True)
        rr_sb = ctx_pool.tile([1, QPC], dt)
        nc.scalar.activation(out=rr_sb[:], in_=rr_ps[:1, :QPC], func=ID,
                             bias=bs2[:])
        nc.sync.dma_start(out=out_rr[:], in_=rr_sb[:])


# ---------------------------------------------------------------------------
# Host-side glue
# ---------------------------------------------------------------------------

def _to2chunk(w):
    return np.ascontiguousarray(w.reshape(2, 128, -1).transpose(1, 0, 2))


def prepare_in_maps(p, inputs):
    f32 = np.float32
    q = np.asarray(inputs["query_emb"], f32)
    E = np.asarray(inputs["index_embs"], f32)
    W_cq = np.asarray(inputs["W_cq"], f32)
    b_cq = np.asarray(inputs["b_cq"], f32)
    W_cd = np.asarray(inputs["W_cd"], f32)
    b_cd = np.asarray(inputs["b_cd"], f32)
    ipw = np.asarray(inputs["in_proj_w"], f32)
    ipb = np.asarray(inputs["in_proj_b"], f32)
    Wq, Wk, Wv = ipw[:p.D], ipw[p.D:2 * p.D], ipw[2 * p.D:]
    bq, bk, bv = ipb[:p.D], ipb[p.D:2 * p.D], ipb[2 * p.D:]
    W_o = np.asarray(inputs["out_proj_w"], f32)
    b_o = np.asarray(inputs["out_proj_b"], f32)
    W_s1 = np.asarray(inputs["W_s1"], f32)
    b_s1 = np.asarray(inputs["b_s1"], f32)
    W_s2 = np.asarray(inputs["W_s2"], f32)
    b_s2 = np.asarray(inputs["b_s2"], f32)

    qT = _to2chunk(np.ascontiguousarray(q.T))
    heads = (np.arange(p.D) // p.hd)
    hmask = np.zeros((p.D, 8), f32)
    hmask[np.arange(p.D), heads] = 1.0
    hmask = _to2chunk(hmask)
    ident = np.eye(128, dtype=f32)
    wcd = np.ascontiguousarray(
        W_cd.reshape(2, 128, 2, 128).transpose(1, 0, 2, 3))
    wk = np.ascontiguousarray(
        Wk.reshape(2, 128, 2, 128).transpose(1, 0, 2, 3))

    def T2(w):
        return _to2chunk(np.ascontiguousarray(w.T))

    def bvec(b):
        return np.ascontiguousarray(b.reshape(2, 128, 1).transpose(1, 0, 2))

    common = dict(
        qT=fp32r_round(qT), eFull=E, hmask=hmask, ident=ident,
        Wcd=wcd, Wk=wk, WkT=T2(Wk), WvT=T2(Wv), WcqT=T2(W_cq), WqT=T2(Wq),
        WoT=T2(W_o), Ws1T=T2(W_s1), Ws2T=np.ascontiguousarray(W_s2.T),
        b_cq=bvec(b_cq), b_cd=bvec(b_cd), b_q=bvec(bq), b_k=bvec(bk),
        b_v=bvec(bv), b_o=bvec(b_o),
        b_s1=np.ascontiguousarray(b_s1.reshape(p.dh, 1)),
        b_s2=np.ascontiguousarray(b_s2.reshape(1, 1)),
    )
    in_maps = []
    for c in range(p.ncores):
        esh = E[c * p.nshard:(c + 1) * p.nshard]
        eTc = np.zeros((p.D, p.npad), f32)
        eTc[:, :p.nshard] = esh.T
        base = np.zeros((128, p.cand), np.uint32)
        for ch in range(p.nch):
            base[:, ch * 8:(ch + 1) * 8] = c * p.nshard + ch * p.chunk
        qbase = np.broadcast_to(
            (np.arange(p.qpc, dtype=np.float64) * p.cand).astype(f32),
            (128, p.qpc)).copy()
        qio = np.broadcast_to(
            np.arange(p.qpc, dtype=f32), (128, p.qpc)).copy()
        m = dict(common)
        m["eT"] = fp32r_round(_to2chunk(eTc))
        m["base_add"] = base
        m["qT_mine"] = np.ascontiguousarray(qT[:, :, c * p.qpc:(c + 1) * p.qpc])
        m["qbaseT"] = qbase
        m["qiota"] = qio
        in_maps.append(m)
    return in_maps


_CACHE = {}


def _get_nc(p):
    key = (p.B, p.N, p.chunk)
    if key not in _CACHE:
        _CACHE[key] = build_nc(p)
    return _CACHE[key]


def run(inputs, trace=False, **kw):
    from concourse.bass_utils import run_bass_kernel_spmd
    p = make_params()
    nc = _get_nc(p)
    in_maps = prepare_in_maps(p, inputs)
    res = run_bass_kernel_spmd(nc, in_maps, core_ids=list(range(p.ncores)),
                               trace=trace, **kw)
    outs = res.results
    top_idx = np.concatenate([o["top_idx"] for o in outs], axis=0)
    top_scores = np.concatenate([o["top_scores"] for o in outs], axis=0)
    rerank = np.concatenate([o["rerank"].reshape(-1) for o in outs], axis=0)
    return (top_idx.astype(np.int32), top_scores, rerank), res


def kernel(**inputs):
    out, _ = run(inputs, trace=False)
    return out


# revision 51
# speedup vs baseline: 1.1623x; 1.0061x over previous
"""Trainium2 distributed kernel for nn_AdMatcher (retrieval_knn).

Strategy (8 NeuronCores, SPMD):
  - index_embs sharded row-wise; scoring matmul in float32r (TF32-grade,
    1 cyc/col) -> chunked per-1024-col top-8 via DVE max/max_index.
  - AllToAll exchanges (val, idx) candidate pairs; each core extracts the
    approx global top-128 for its 64 queries (iterative max8/match_replace).
  - Winner (val, idx) pairs and embedding rows gathered via SWDGE indirect
    DMA (one offset per partition); exact fp32 rescore on PE + resort gives
    the exact top-128 order/values; idx permuted through a DRAM scratch.
  - Rerank cross-attention + MLP runs in bf16, query-parallel, with the
    kh projection folded into the query side (G = M_k^T @ QhS).

kernel(**inputs) returns (top_idx [512,128] i32, top_scores [512,128] f32,
rerank_scores [512] f32).
"""

import math
import numpy as np

from concourse import bass, bacc, mybir, tile

FP32 = mybir.dt.float32
FP32R = mybir.dt.float32r
BF16 = mybir.dt.bfloat16
U32 = mybir.dt.uint32
I32 = mybir.dt.int32

NEG = -1.0e30
ID = mybir.ActivationFunctionType.Identity
EXP = mybir.ActivationFunctionType.Exp
RELU = mybir.ActivationFunctionType.Relu


def _p(**kw):
    class P:
        pass

    p = P()
    for k, v in kw.items():
        setattr(p, k, v)
    return p


def make_params(B=512, D=256, N=200000, K=128, ncores=8, chunk=1024):
    nshard = N // ncores
    nch = math.ceil(nshard / chunk)
    return _p(
        B=B, D=D, N=N, K=K, ncores=ncores, chunk=chunk,
        nshard=nshard, nch=nch, npad=nch * chunk, qpc=B // ncores,
        cand=nch * 8, mw=ncores * nch * 8,
        nqb=max(B // 128, 1), qb_size=min(B, 128),
        nheads=8, hd=D // 8, dh=D // 2,
    )


def fp32r_round(a):
    """Host-side float32r rounding (round-to-nearest, 11-bit mantissa)."""
    b = np.ascontiguousarray(a, np.float32).view(np.uint32)
    r = (b + np.uint32(0x800)) & np.uint32(0xFFFFF000)
    return r.view(np.float32)


# ---------------------------------------------------------------------------
# Builder
# ---------------------------------------------------------------------------

def build_nc(p):
    nc = bacc.Bacc("TRN2", target_bir_lowering=False, debug=False,
                   num_devices=p.ncores)
    dt = FP32
    KD = 2
    QB, NQB = p.qb_size, p.nqb
    CH, NCH, CAND = p.chunk, p.nch, p.cand
    QPC, MW = p.qpc, p.mw
    ROWW = 2 * CAND

    def din(name, shape, dtype=dt):
        return nc.dram_tensor(name, shape, dtype, kind="ExternalInput")

    qT = din("qT", [128, KD, p.B], FP32R)
    qT_mine = din("qT_mine", [128, KD, QPC])
    eT = din("eT", [128, KD, p.npad], FP32R)
    eFull = din("eFull", [p.N, p.D])
    base_add = din("base_add", [128, CAND], U32)
    qbaseT = din("qbaseT", [128, QPC])       # i*CAND
    qiota = din("qiota", [128, QPC])         # i
    hmask = din("hmask", [128, KD, 8])
    ident = din("ident", [128, 128])
    Wcd = din("Wcd", [128, KD, KD, 128])
    Wk = din("Wk", [128, KD, KD, 128])       # M_k needs untransposed chunks
    WkT = din("WkT", [128, KD, p.D])
    WvT = din("WvT", [128, KD, p.D])
    WcqT = din("WcqT", [128, KD, p.D])
    WqT = din("WqT", [128, KD, p.D])
    WoT = din("WoT", [128, KD, p.D])
    Ws1T = din("Ws1T", [128, KD, p.dh])
    Ws2T = din("Ws2T", [128, 1])
    b_cq = din("b_cq", [128, KD, 1])
    b_cd = din("b_cd", [128, KD, 1])
    b_q = din("b_q", [128, KD, 1])
    b_k = din("b_k", [128, KD, 1])
    b_v = din("b_v", [128, KD, 1])
    b_o = din("b_o", [128, KD, 1])
    b_s1 = din("b_s1", [p.dh, 1])
    b_s2 = din("b_s2", [1, 1])

    out_idx = nc.dram_tensor("top_idx", [QPC, p.K], I32, kind="ExternalOutput")
    out_scores = nc.dram_tensor("top_scores", [QPC, p.K], dt,
                                kind="ExternalOutput")
    out_rr = nc.dram_tensor("rerank", [1, QPC], dt, kind="ExternalOutput")

    a2a_in = nc.dram_tensor("a2a_in", [p.B, ROWW], dt)
    a2a_out = nc.dram_tensor("a2a_out", [p.B, ROWW], dt)
    idx_scratch = nc.dram_tensor("idx_scratch", [p.K * QPC, 64], U32)

    with tile.TileContext(nc) as tc:
        # ================= Stage A: scoring + per-chunk top-8 =============
        with tc.tile_pool(name="qt", bufs=1) as qt_pool, \
             tc.tile_pool(name="et", bufs=3) as et_pool, \
             tc.tile_pool(name="psA", bufs=4, space="PSUM") as psA, \
             tc.tile_pool(name="candp", bufs=1) as cand_pool, \
             tc.tile_pool(name="miscA", bufs=1) as miscA:

            qT_sb = qt_pool.tile([128, KD, p.B], FP32R)
            nc.sync.dma_start(out=qT_sb[:], in_=qT[:])
            base_sb = miscA.tile([128, CAND], U32)
            nc.sync.dma_start(out=base_sb[:], in_=base_add[:])

            packs = [cand_pool.tile([QB, ROWW], U32, tag=f"pack{qb}",
                                    name=f"pack{qb}")
                     for qb in range(NQB)]

            for c in range(NCH):
                et_tile = et_pool.tile([128, KD, CH], FP32R, tag="et")
                nc.sync.dma_start(out=et_tile[:],
                                  in_=eT[:, :, c * CH:(c + 1) * CH])
                for qb in range(NQB):
                    ps = psA.tile([128, CH], FP32, tag="psA")
                    mmw = min(CH, 512)
                    for h in range(CH // mmw):
                        for kc in range(KD):
                            nc.tensor.matmul(
                                out=ps[:QB, h * mmw:(h + 1) * mmw],
                                lhsT=qT_sb[:, kc, qb * QB:(qb + 1) * QB],
                                rhs=et_tile[:, kc, h * mmw:(h + 1) * mmw],
                                start=(kc == 0), stop=(kc == KD - 1))
                    pv = packs[qb][:].rearrange("q (c t) -> q c t", t=2)
                    vals8 = pv[:, c * 8:(c + 1) * 8, 0].bitcast(FP32)
                    nc.vector.max(out=vals8, in_=ps[:QB, :])
                    nc.vector.max_index(
                        out=pv[:, c * 8:(c + 1) * 8, 1],
                        in_max=vals8, in_values=ps[:QB, :])
            for qb in range(NQB):
                pv = packs[qb][:].rearrange("q (c t) -> q c t", t=2)
                nc.vector.tensor_tensor(
                    out=pv[:, :, 1], in0=pv[:, :, 1],
                    in1=base_sb[:QB, :], op=mybir.AluOpType.add)
                nc.sync.dma_start(
                    out=a2a_in[qb * QB:(qb + 1) * QB, :].bitcast(U32),
                    in_=packs[qb][:])

            nc.gpsimd.collective_compute(
                "AllToAll", mybir.AluOpType.bypass,
                replica_groups=[list(range(p.ncores))],
                ins=[a2a_in[:].opt()], outs=[a2a_out[:].opt()])

        # ================= Stage C: global merge ==========================
        with tc.tile_pool(name="mrg", bufs=1) as mrg_pool, \
             tc.tile_pool(name="sel", bufs=1) as sel_pool, \
             tc.tile_pool(name="pst", bufs=1, space="PSUM") as pst:
            # contiguous load of all (val, idx) pairs for my queries;
            # the extraction scans the stride-2 value view in place.
            pairs_sb = mrg_pool.tile([QPC, p.ncores, 2 * CAND], dt)
            nc.sync.dma_start(
                out=pairs_sb[:],
                in_=a2a_out.ap().rearrange("(s i) w -> i s w", s=p.ncores))
            vals_sb = mrg_pool.tile([QPC, MW], dt)
            nc.vector.tensor_copy(
                out=vals_sb[:].rearrange("i (s w) -> i s w", s=p.ncores),
                in_=pairs_sb[:].rearrange("i s (w t) -> i s w t",
                                          t=2)[:, :, :, 0])
            work = vals_sb[:]

            appr_sb = sel_pool.tile([QPC, p.K], dt)
            slots_sb = sel_pool.tile([QPC, p.K], U32)
            nrounds = p.K // 8
            for r in range(nrounds):
                v8 = appr_sb[:, r * 8:(r + 1) * 8]
                nc.vector.max(out=v8, in_=work)
                nc.vector.max_index(out=slots_sb[:, r * 8:(r + 1) * 8],
                                    in_max=v8, in_values=work)
                if r != nrounds - 1:
                    nc.vector.match_replace(out=work, in_to_replace=v8,
                                            in_values=work, imm_value=NEG)

            # ---- slot -> pair index (rank-on-partition orientation) ----
            inv = np.float32(1.0) / np.float32(CAND)
            MAGIC = np.float32(12582912.0)
            FEPS = np.float32(0.4985)
            fl = np.float32(np.float32(np.float32(
                np.arange(MW, dtype=np.float32) * inv) - FEPS) + MAGIC) - MAGIC
            assert np.all(fl == (np.arange(MW) // CAND)), "floor trick invalid"

            id_sb = sel_pool.tile([128, 128], dt)
            nc.sync.dma_start(out=id_sb[:], in_=ident[:])
            qb_sb = sel_pool.tile([128, QPC], dt)
            nc.sync.dma_start(out=qb_sb[:], in_=qbaseT[:])
            sf = sel_pool.tile([QPC, p.K], dt)
            nc.vector.tensor_copy(out=sf[:], in_=slots_sb[:])
            tp = pst.tile([128, 128], FP32, tag="tp")
            nc.tensor.transpose(out=tp[:, :QPC], in_=sf[:],
                                identity=id_sb[:QPC, :QPC])
            sfT = sel_pool.tile([p.K, QPC], dt)
            nc.scalar.activation(out=sfT[:], in_=tp[:p.K, :QPC], func=ID)
            segf = sel_pool.tile([p.K, QPC], dt)
            nc.vector.tensor_scalar(out=segf[:], in0=sfT[:],
                                    scalar1=float(inv), scalar2=float(FEPS),
                                    op0=mybir.AluOpType.mult,
                                    op1=mybir.AluOpType.subtract)
            nc.vector.tensor_scalar(out=segf[:], in0=segf[:],
                                    scalar1=float(MAGIC), scalar2=float(MAGIC),
                                    op0=mybir.AluOpType.add,
                                    op1=mybir.AluOpType.subtract)
            nc.vector.tensor_scalar(out=segf[:], in0=segf[:],
                                    scalar1=float(CAND * (QPC - 1)),
                                    scalar2=None, op0=mybir.AluOpType.mult)
            nc.vector.tensor_add(sfT[:], sfT[:], segf[:])
            nc.vector.tensor_tensor(out=sfT[:], in0=sfT[:],
                                    in1=qb_sb[:p.K, :],
                                    op=mybir.AluOpType.add)
            flat_u = sel_pool.tile([p.K, QPC], U32)
            nc.vector.tensor_copy(out=flat_u[:], in_=sfT[:])

            # ---- per-query gathers: (val, idx) pair then embedding row ----
            a2a_pairs = a2a_out.ap().rearrange("b (w t) -> (b w) t", t=2)
            pairbuf = sel_pool.tile([p.K, QPC, 2], dt)
            cand_all = sel_pool.tile([p.K, QPC, p.D], dt)
            for i in range(QPC):
                nc.gpsimd.indirect_dma_start(
                    out=pairbuf[:, i, :], out_offset=None,
                    in_=a2a_pairs,
                    in_offset=bass.IndirectOffsetOnAxis(
                        ap=flat_u[:, i:i + 1], axis=0))
                nc.gpsimd.indirect_dma_start(
                    out=cand_all[:, i, :], out_offset=None,
                    in_=eFull[:],
                    in_offset=bass.IndirectOffsetOnAxis(
                        ap=pairbuf[:, i, 1:2].bitcast(U32), axis=0))

            # stash gathered ids for the final permutation (col 0 of each
            # 256-byte row so dma_gather's row-size constraint is met)
            nc.sync.dma_start(
                out=idx_scratch[:, 0:1],
                in_=pairbuf[:, :, 1].bitcast(U32).unsqueeze(2))

            _rerank(nc, tc, p, cand_all, pairbuf, idx_scratch, qT_mine,
                    WcqT, WqT, Wcd, Wk, WkT, WvT, WoT, Ws1T, Ws2T, b_cq,
                    b_cd, b_q, b_k, b_v, b_o, b_s1, b_s2, hmask, qiota,
                    id_sb, pst, out_idx, out_scores, out_rr)

    nc.compile()
    return nc


def _rerank(nc, tc, p, cand_all, pairbuf, idx_scratch, qT_mine, WcqT, WqT,
            Wcd, Wk, WkT, WvT, WoT, Ws1T, Ws2T, b_cq, b_cd, b_q, b_k, b_v,
            b_o, b_s1, b_s2, hmask, qiota, id_sb, pst, out_idx, out_scores,
            out_rr):
    dt = FP32
    KD = 2
    QPC = p.qpc

    with tc.tile_pool(name="wts", bufs=1) as wp, \
         tc.tile_pool(name="cT", bufs=6) as cT_pool, \
         tc.tile_pool(name="kvV", bufs=17) as kvV, \
         tc.tile_pool(name="at", bufs=4) as at_pool, \
         tc.tile_pool(name="ctxp", bufs=1) as ctx_pool, \
         tc.tile_pool(name="psB", bufs=3, space="PSUM") as psB, \
         tc.tile_pool(name="psSm", bufs=2, space="PSUM") as psSm:

        def big_ps():
            return psB.tile([128, 512], FP32, tag="big", name="bigps")

        def ldb(t, shape):
            """Load a weight tensor, casting fp32 -> bf16 in the DMA."""
            s = wp.tile(shape, BF16, tag=t.name, name="w_" + t.name)
            nc.gpsimd.dma_start(out=s[:], in_=t[:])
            return s

        def ldf(t, shape):
            s = wp.tile(shape, dt, tag=t.name, name="w_" + t.name)
            nc.sync.dma_start(out=s[:], in_=t[:])
            return s

        wcd = ldb(Wcd, [128, KD, KD, 128])
        wkT = ldb(WkT, [128, KD, p.D])
        wvT = ldb(WvT, [128, KD, p.D])
        wcqT = ldb(WcqT, [128, KD, p.D])
        wqT = ldb(WqT, [128, KD, p.D])
        woT = ldb(WoT, [128, KD, p.D])
        ws1T = ldb(Ws1T, [128, KD, p.dh])
        ws2T = ldb(Ws2T, [128, 1])
        bcq = ldf(b_cq, [128, KD, 1])
        bcd = ldf(b_cd, [128, KD, 1])
        bq = ldf(b_q, [128, KD, 1])
        bk = ldf(b_k, [128, KD, 1])
        bv = ldf(b_v, [128, KD, 1])
        bo = ldf(b_o, [128, KD, 1])
        bs1 = ldf(b_s1, [p.dh, 1])
        bs2 = ldf(b_s2, [1, 1])
        hm = ldb(hmask, [128, KD, 8])
        hmf = ldf(hmask, [128, KD, 8])
        qtm = ldf(qT_mine, [128, KD, QPC])
        qtm_b = wp.tile([128, KD, QPC], BF16, tag="qtmb")
        nc.vector.tensor_copy(out=qtm_b[:], in_=qtm[:])
        qio = ldf(qiota, [128, QPC])

        # M_vT (bf16): [128(j), jc, i] = (Wv @ Wcd)[i, jc*128+j]
        mvT = wp.tile([128, KD, p.D], BF16, tag="mvT")
        for jc in range(KD):
            ps = big_ps()
            for lc in range(KD):
                nc.tensor.matmul(out=ps[:, :p.D], lhsT=wcd[:, lc, jc, :],
                                 rhs=wvT[:, lc, :],
                                 start=(lc == 0), stop=(lc == KD - 1))
            nc.scalar.activation(out=mvT[:, jc, :], in_=ps[:, :p.D], func=ID)
        # M_k untransposed chunks (bf16): mk[128(j), jc(j), ic(j'), 128(j')]
        #   = M_k[jc*128+j, ic*128+j'] = sum_l Wk[j, l] Wcd[l, j']
        mk = wp.tile([128, KD, KD, 128], BF16, tag="mk")
        for jc in range(KD):
            ps = big_ps()
            for lc in range(KD):
                # out[j, j'] = sum_l WkT[l, jc*128+j]^T ... lhsT = wkT chunk
                nc.tensor.matmul(out=ps[:, :p.D],
                                 lhsT=wkT[:, lc, jc * 128:(jc + 1) * 128],
                                 rhs=wcd[:, lc, :, :].rearrange("l a b -> l (a b)"),
                                 start=(lc == 0), stop=(lc == KD - 1))
            nc.scalar.activation(
                out=mk[:, jc, :, :].rearrange("j a b -> j (a b)"),
                in_=ps[:, :p.D], func=ID)

        # c_k = Wk @ b_cd + bk ; c_v = Wv @ b_cd + bv
        bcd_b = wp.tile([128, KD, 1], BF16, tag="bcdb")
        nc.vector.tensor_copy(out=bcd_b[:], in_=bcd[:])
        ck = wp.tile([128, KD, 1], dt, tag="ck")
        cv = wp.tile([128, KD, 1], dt, tag="cv")
        for (ct, wt, bt) in ((ck, wkT, bk), (cv, wvT, bv)):
            for m in range(KD):
                ps = psSm.tile([128, 8], FP32, tag="small", name="smps")
                for jc in range(KD):
                    nc.tensor.matmul(out=ps[:, :1],
                                     lhsT=wt[:, jc, m * 128:(m + 1) * 128],
                                     rhs=bcd_b[:, jc, :],
                                     start=(jc == 0), stop=(jc == KD - 1))
                nc.scalar.activation(out=ct[:, m, :], in_=ps[:, :1], func=ID,
                                     bias=bt[:, m, :])

        # Q'T = W_cq @ qT_mine + b_cq ; qhT = (Wq @ Q'T + bq) / sqrt(hd)
        qpT = wp.tile([128, KD, QPC], BF16, tag="qpT")
        qhT = wp.tile([128, KD, QPC], dt, tag="qhT")
        for m in range(KD):
            ps = big_ps()
            for jc in range(KD):
                nc.tensor.matmul(out=ps[:, :QPC],
                                 lhsT=wcqT[:, jc, m * 128:(m + 1) * 128],
                                 rhs=qtm_b[:, jc, :], start=(jc == 0),
                                 stop=(jc == KD - 1))
            nc.scalar.activation(out=qpT[:, m, :], in_=ps[:, :QPC], func=ID,
                                 bias=bcq[:, m, :])
        sc = 1.0 / math.sqrt(p.hd)
        bq_s = wp.tile([128, KD, 1], dt, tag="bqs")
        nc.vector.tensor_scalar(out=bq_s[:], in0=bq[:], scalar1=sc,
                                scalar2=None, op0=mybir.AluOpType.mult)
        for m in range(KD):
            ps = big_ps()
            for jc in range(KD):
                nc.tensor.matmul(out=ps[:, :QPC],
                                 lhsT=wqT[:, jc, m * 128:(m + 1) * 128],
                                 rhs=qpT[:, jc, :], start=(jc == 0),
                                 stop=(jc == KD - 1))
            nc.scalar.activation(out=qhT[:, m, :], in_=ps[:, :QPC], func=ID,
                                 bias=bq_s[:, m, :], scale=sc)

        # QhS_all[j, jc, q, h] = qhT * hmask ; G = M_k^T @ QhS (bf16)
        qhs_all = wp.tile([128, KD, QPC, 8], BF16, tag="qhs")
        nc.vector.tensor_tensor(
            out=qhs_all[:],
            in0=qhT[:].unsqueeze(3).to_broadcast([128, KD, QPC, 8]),
            in1=hmf[:].unsqueeze(2).to_broadcast([128, KD, QPC, 8]),
            op=mybir.AluOpType.mult)
        g_all = wp.tile([128, KD, QPC, 8], BF16, tag="g_all")
        for jpc in range(KD):
            nmm = (QPC * 8) // 512
            for b in range(max(nmm, 1)):
                w = min(512, QPC * 8)
                ps = big_ps()
                for jc in range(KD):
                    nc.tensor.matmul(
                        out=ps[:, :w],
                        lhsT=mk[:, jc, jpc, :],
                        rhs=qhs_all[:, jc, :, :].rearrange(
                            "j q h -> j (q h)")[:, b * 512:b * 512 + w],
                        start=(jc == 0), stop=(jc == KD - 1))
                nc.scalar.activation(
                    out=g_all[:, jpc, :, :].rearrange(
                        "j q h -> j (q h)")[:, b * 512:b * 512 + w],
                    in_=ps[:, :w], func=ID)

        # ---- per-query: transpose cand, rescore, attention ----
        sexT = ctx_pool.tile([p.K, QPC], dt)
        ctxcol = ctx_pool.tile([128, KD, QPC, 8], dt)
        nbat = max(QPC // 16, 1)
        bsz = QPC // nbat
        for bi in range(nbat):
            attn = at_pool.tile([8, bsz, 128], dt, tag="attn")
            vhs = []
            for qq in range(bsz):
                q = bi * bsz + qq
                candq = cand_all[:, q, :]
                cTf = cT_pool.tile([128, KD, 128], dt, tag="cTf")
                cTb = cT_pool.tile([128, KD, 128], BF16, tag="cTb")
                for m in range(KD):
                    tpp = big_ps()
                    nc.tensor.transpose(out=tpp[:, :128],
                                        in_=candq[:, m * 128:(m + 1) * 128],
                                        identity=id_sb[:])
                    nc.scalar.activation(out=cTf[:, m, :], in_=tpp[:, :128],
                                         func=ID)
                    nc.scalar.activation(out=cTb[:, m, :], in_=tpp[:, :128],
                                         func=ID)
                # exact rescore: s_ex[k] = sum_d candT[d,k] * q[d]
                ps = psSm.tile([128, 8], FP32, tag="small", name="smps2")
                for jc in range(KD):
                    nc.tensor.matmul(out=ps[:, :1], lhsT=cTf[:, jc, :],
                                     rhs=qtm[:, jc, q:q + 1],
                                     start=(jc == 0), stop=(jc == KD - 1))
                nc.scalar.activation(out=sexT[:, q:q + 1], in_=ps[:, :1],
                                     func=ID)
                # vh = cand @ M_v^T + (c_v deferred)  [128k, D] bf16
                vh = kvV.tile([128, p.D], BF16, tag="vh")
                ps2 = big_ps()
                for jc in range(KD):
                    nc.tensor.matmul(out=ps2[:, :p.D], lhsT=cTb[:, jc, :],
                                     rhs=mvT[:, jc, :],
                                     start=(jc == 0), stop=(jc == KD - 1))
                nc.scalar.activation(out=vh[:], in_=ps2[:, :p.D], func=ID)
                vhs.append(vh)
                # logits = G_q^T @ candT
                lg_ps = psSm.tile([8, 128], FP32, tag="lgps", name="lgps")
                for jpc in range(KD):
                    nc.tensor.matmul(out=lg_ps[:],
                                     lhsT=g_all[:, jpc, q, :],
                                     rhs=cTb[:, jpc, :],
                                     start=(jpc == 0), stop=(jpc == KD - 1))
                nc.scalar.activation(out=attn[:, qq, :], in_=lg_ps[:],
                                     func=ID)
            nmax = at_pool.tile([8, bsz], dt, tag="nmax")
            nc.vector.tensor_reduce(out=nmax[:], in_=attn[:],
                                    axis=mybir.AxisListType.X,
                                    op=mybir.AluOpType.max, negate=True)
            nc.vector.tensor_tensor(
                out=attn[:], in0=attn[:],
                in1=nmax[:].unsqueeze(2).to_broadcast([8, bsz, 128]),
                op=mybir.AluOpType.add)
            nc.scalar.activation(out=attn[:], in_=attn[:], func=EXP)
            ssum = at_pool.tile([8, bsz], dt, tag="ssum")
            nc.vector.tensor_reduce(out=ssum[:], in_=attn[:],
                                    axis=mybir.AxisListType.X,
                                    op=mybir.AluOpType.add)
            rinv = at_pool.tile([8, bsz], dt, tag="rinv")
            nc.vector.reciprocal(out=rinv[:], in_=ssum[:])
            nc.vector.tensor_tensor(
                out=attn[:], in0=attn[:],
                in1=rinv[:].unsqueeze(2).to_broadcast([8, bsz, 128]),
                op=mybir.AluOpType.mult)
            for qq in range(bsz):
                q = bi * bsz + qq
                tpp = psSm.tile([128, 8], FP32, tag="small", name="smps3")
                nc.tensor.transpose(out=tpp[:], in_=attn[:, qq, :],
                                    identity=id_sb[:8, :8])
                attnT = at_pool.tile([128, 8], BF16, tag="attnT")
                nc.scalar.activation(out=attnT[:], in_=tpp[:], func=ID)
                for m in range(KD):
                    ps3 = psSm.tile([128, 8], FP32, tag="small", name="smps4")
                    nc.tensor.matmul(out=ps3[:],
                                     lhsT=vhs[qq][:, m * 128:(m + 1) * 128],
                                     rhs=attnT[:], start=True, stop=True)
                    nc.scalar.activation(out=ctxcol[:, m, q, :], in_=ps3[:],
                                         func=ID, bias=cv[:, m, :])

        # ---- exact resort ----
        tp = pst.tile([128, 128], FP32, tag="tp")
        nc.tensor.transpose(out=tp[:QPC, :p.K], in_=sexT[:],
                            identity=id_sb[:])
        sex = ctx_pool.tile([QPC, p.K], dt)
        nc.scalar.activation(out=sex[:], in_=tp[:QPC, :p.K], func=ID)
        swork = ctx_pool.tile([QPC, p.K], dt)
        nc.vector.tensor_copy(out=swork[:], in_=sex[:])
        scr_sb = ctx_pool.tile([QPC, p.K], dt)
        slots3 = ctx_pool.tile([QPC, p.K], U32)
        nr = p.K // 8
        for r in range(nr):
            v8 = scr_sb[:, r * 8:(r + 1) * 8]
            nc.vector.max(out=v8, in_=swork[:])
            nc.vector.max_index(out=slots3[:, r * 8:(r + 1) * 8],
                                in_max=v8, in_values=swork[:])
            if r != nr - 1:
                nc.vector.match_replace(out=swork[:], in_to_replace=v8,
                                        in_values=swork[:], imm_value=NEG)
        nc.sync.dma_start(out=out_scores[:], in_=scr_sb[:])

        # ---- permute ids by exact rank via DRAM scratch ----
        # pipelined by rank halves: the dma_gather for ranks [0,64) runs
        # while the resort rounds for ranks [64,128) still execute.
        for hh in range(2):
            j0 = hh * (p.K // 2)
            s3f = ctx_pool.tile([QPC, p.K // 2], dt, tag="s3f", bufs=2,
                                name=f"s3f{hh}")
            nc.vector.tensor_copy(out=s3f[:],
                                  in_=slots3[:, j0:j0 + p.K // 2])
            tp2 = pst.tile([128, 128], FP32, tag="tp", name=f"tp2{hh}")
            nc.tensor.transpose(out=tp2[:p.K // 2, :QPC], in_=s3f[:],
                                identity=id_sb[:QPC, :QPC])
            f3T = ctx_pool.tile([p.K // 2, QPC], dt, tag="f3T", bufs=2,
                                name=f"f3T{hh}")
            nc.scalar.activation(out=f3T[:], in_=tp2[:p.K // 2, :QPC],
                                 func=ID, scale=float(QPC))
            nc.vector.tensor_tensor(out=f3T[:], in0=f3T[:],
                                    in1=qio[:p.K // 2, :],
                                    op=mybir.AluOpType.add)
            f3i = ctx_pool.tile([p.K // 2, QPC], mybir.dt.int16, tag="f3i", bufs=2,
                                name=f"f3i{hh}")
            nc.vector.tensor_copy(out=f3i[:], in_=f3T[:])
            # idx list order k = q*(K/2) + j -> wrapped[(j%16), q*(K/32)+j//16]
            nh = p.K // 32          # hi values per half (4 for K=128)
            wrapped = ctx_pool.tile([128, QPC * nh], mybir.dt.int16,
                                    tag="wrap", bufs=2, name=f"wrap{hh}")
            wv = wrapped[:].rearrange("pp (q h) -> pp q h", h=nh)
            for hi in range(nh):
                nc.sync.dma_start(out=wv[:16, :, hi],
                                  in_=f3i[hi * 16:(hi + 1) * 16, :])
            for g in range(1, 8):
                nc.sync.dma_start(out=wrapped[g * 16:(g + 1) * 16, :],
                                  in_=wrapped[:16, :])
            nidx = (p.K // 2) * QPC
            gbuf = ctx_pool.tile([128, nidx // 128, 64], dt, tag="gbuf", bufs=2,
                                 name=f"gbuf{hh}")
            nc.gpsimd.dma_gather(
                out_ap=gbuf[:], in_ap=idx_scratch.ap().bitcast(FP32),
                idxs_ap=wrapped[:],
                num_idxs=nidx, num_idxs_reg=nidx, elem_size=64,
                single_packet=False)
            # out[p, b] = idx(q = 2b + p//64, j = p%64): transpose, then
            # rows b hold [q=2b ranks | q=2b+1 ranks]
            gf = ctx_pool.tile([128, nidx // 128], dt, tag="gf", bufs=2,
                               name=f"gf{hh}")
            nc.vector.tensor_copy(out=gf[:], in_=gbuf[:, :, 0].bitcast(U32))
            tp4 = pst.tile([128, 128], FP32, tag="tp", name=f"tp4{hh}")
            nc.tensor.transpose(out=tp4[:nidx // 128, :128], in_=gf[:],
                                identity=id_sb[:])
            tsb = ctx_pool.tile([nidx // 128, 128], I32, tag="tsb", bufs=2,
                                name=f"tsb{hh}")
            nc.vector.tensor_copy(out=tsb[:], in_=tp4[:nidx // 128, :128])
            nc.sync.dma_start(
                out=out_idx.ap()[:, j0:j0 + p.K // 2].rearrange(
                    "(b par) j -> b par j", par=2),
                in_=tsb[:].rearrange("b (par j) -> b par j", par=2))

        # ---- diag extract + output head ----
        ctxTf = ctx_pool.tile([128, KD, QPC], dt)
        tmp = ctx_pool.tile([128, KD, QPC, 8], dt)
        nc.vector.tensor_tensor(
            out=tmp[:], in0=ctxcol[:],
            in1=hmf[:].unsqueeze(2).to_broadcast([128, KD, QPC, 8]),
            op=mybir.AluOpType.mult)
        nc.vector.tensor_reduce(out=ctxTf[:], in_=tmp[:],
                                axis=mybir.AxisListType.X,
                                op=mybir.AluOpType.add)
        ctxT = ctx_pool.tile([128, KD, QPC], BF16)
        nc.vector.tensor_copy(out=ctxT[:], in_=ctxTf[:])

        crossT = ctx_pool.tile([128, KD, QPC], BF16)
        for m in range(KD):
            ps = big_ps()
            for jc in range(KD):
                nc.tensor.matmul(out=ps[:, :QPC],
                                 lhsT=woT[:, jc, m * 128:(m + 1) * 128],
                                 rhs=ctxT[:, jc, :], start=(jc == 0),
                                 stop=(jc == KD - 1))
            nc.scalar.activation(out=crossT[:, m, :], in_=ps[:, :QPC],
                                 func=ID, bias=bo[:, m, :])
        s1T = ctx_pool.tile([p.dh, QPC], BF16)
        ps = big_ps()
        for jc in range(KD):
            nc.tensor.matmul(out=ps[:p.dh, :QPC], lhsT=ws1T[:, jc, :],
                             rhs=crossT[:, jc, :], start=(jc == 0),
                             stop=(jc == KD - 1))
        nc.scalar.activation(out=s1T[:], in_=ps[:p.dh, :QPC], func=RELU,
                             bias=bs1[:])
        rr_ps = big_ps()
        nc.tensor.matmul(out=rr_ps[:1, :QPC], lhsT=ws2T[:p.dh, :],
                         rhs=s1T[:], start=True, stop=True)
        rr_sb = ctx_pool.tile([1, QPC], dt)
        nc.scalar.activation(out=rr_sb[:], in_=rr_ps[:1, :QPC], func=ID,
                             bias=bs2[:])
        nc.sync.dma_start(out=out_rr[:], in_=rr_sb[:])


# ---------------------------------------------------------------------------
# Host-side glue
# ---------------------------------------------------------------------------

def _to2chunk(w):
    return np.ascontiguousarray(w.reshape(2, 128, -1).transpose(1, 0, 2))


def prepare_in_maps(p, inputs):
    f32 = np.float32
    q = np.asarray(inputs["query_emb"], f32)
    E = np.asarray(inputs["index_embs"], f32)
    W_cq = np.asarray(inputs["W_cq"], f32)
    b_cq = np.asarray(inputs["b_cq"], f32)
    W_cd = np.asarray(inputs["W_cd"], f32)
    b_cd = np.asarray(inputs["b_cd"], f32)
    ipw = np.asarray(inputs["in_proj_w"], f32)
    ipb = np.asarray(inputs["in_proj_b"], f32)
    Wq, Wk, Wv = ipw[:p.D], ipw[p.D:2 * p.D], ipw[2 * p.D:]
    bq, bk, bv = ipb[:p.D], ipb[p.D:2 * p.D], ipb[2 * p.D:]
    W_o = np.asarray(inputs["out_proj_w"], f32)
    b_o = np.asarray(inputs["out_proj_b"], f32)
    W_s1 = np.asarray(inputs["W_s1"], f32)
    b_s1 = np.asarray(inputs["b_s1"], f32)
    W_s2 = np.asarray(inputs["W_s2"], f32)
    b_s2 = np.asarray(inputs["b_s2"], f32)

    qT = _to2chunk(np.ascontiguousarray(q.T))
    heads = (np.arange(p.D) // p.hd)
    hmask = np.zeros((p.D, 8), f32)
    hmask[np.arange(p.D), heads] = 1.0
    hmask = _to2chunk(hmask)
    ident = np.eye(128, dtype=f32)
    wcd = np.ascontiguousarray(
        W_cd.reshape(2, 128, 2, 128).transpose(1, 0, 2, 3))
    wk = np.ascontiguousarray(
        Wk.reshape(2, 128, 2, 128).transpose(1, 0, 2, 3))

    def T2(w):
        return _to2chunk(np.ascontiguousarray(w.T))

    def bvec(b):
        return np.ascontiguousarray(b.reshape(2, 128, 1).transpose(1, 0, 2))

    common = dict(
        qT=fp32r_round(qT), eFull=E, hmask=hmask, ident=ident,
        Wcd=wcd, Wk=wk, WkT=T2(Wk), WvT=T2(Wv), WcqT=T2(W_cq), WqT=T2(Wq),
        WoT=T2(W_o), Ws1T=T2(W_s1), Ws2T=np.ascontiguousarray(W_s2.T),
        b_cq=bvec(b_cq), b_cd=bvec(b_cd), b_q=bvec(bq), b_k=bvec(bk),
        b_v=bvec(bv), b_o=bvec(b_o),
        b_s1=np.ascontiguousarray(b_s1.reshape(p.dh, 1)),
        b_s2=np.ascontiguousarray(b_s2.reshape(1, 1)),
    )
    in_maps = []
    for c in range(p.ncores):
        esh = E[c * p.nshard:(c + 1) * p.nshard]
        eTc = np.zeros((p.D, p.npad), f32)
        eTc[:, :p.nshard] = esh.T
        base = np.zeros((128, p.cand), np.uint32)
        for ch in range(p.nch):
            base[:, ch * 8:(ch + 1) * 8] = c * p.nshard + ch * p.chunk
        qbase = np.broadcast_to(
            (np.arange(p.qpc, dtype=np.float64) * p.cand).astype(f32),
            (128, p.qpc)).copy()
        qio = np.broadcast_to(
            np.arange(p.qpc, dtype=f32), (128, p.qpc)).copy()
        m = dict(common)
        m["eT"] = fp32r_round(_to2chunk(eTc))
        m["base_add"] = base
        m["qT_mine"] = np.ascontiguousarray(qT[:, :, c * p.qpc:(c + 1) * p.qpc])
        m["qbaseT"] = qbase
        m["qiota"] = qio
        in_maps.append(m)
    return in_maps


_CACHE = {}


def _get_nc(p):
    key = (p.B, p.N, p.chunk)
    if key not in _CACHE:
        _CACHE[key] = build_nc(p)
    return _CACHE[key]


def run(inputs, trace=False, **kw):
    from concourse.bass_utils import run_bass_kernel_spmd
    p = make_params()
    nc = _get_nc(p)
    in_maps = prepare_in_maps(p, inputs)
    res = run_bass_kernel_spmd(nc, in_maps, core_ids=list(range(p.ncores)),
                               trace=trace, **kw)
    outs = res.results
    top_idx = np.concatenate([o["top_idx"] for o in outs], axis=0)
    top_scores = np.concatenate([o["top_scores"] for o in outs], axis=0)
    rerank = np.concatenate([o["rerank"].reshape(-1) for o in outs], axis=0)
    return (top_idx.astype(np.int32), top_scores, rerank), res


def kernel(**inputs):
    out, _ = run(inputs, trace=False)
    return out


# revision 53
# speedup vs baseline: 1.1697x; 1.0064x over previous
"""Trainium2 distributed kernel for nn_AdMatcher (retrieval_knn).

Strategy (8 NeuronCores, SPMD):
  - index_embs sharded row-wise; scoring matmul in float32r (TF32-grade,
    1 cyc/col) -> chunked per-1024-col top-8 via DVE max/max_index.
  - AllToAll exchanges (val, idx) candidate pairs; each core extracts the
    approx global top-128 for its 64 queries (iterative max8/match_replace).
  - Winner (val, idx) pairs and embedding rows gathered via SWDGE indirect
    DMA (one offset per partition); exact fp32 rescore on PE + resort gives
    the exact top-128 order/values; idx permuted through a DRAM scratch.
  - Rerank cross-attention + MLP runs in bf16, query-parallel, with the
    kh projection folded into the query side (G = M_k^T @ QhS).

kernel(**inputs) returns (top_idx [512,128] i32, top_scores [512,128] f32,
rerank_scores [512] f32).
"""

import math
import numpy as np

from concourse import bass, bacc, mybir, tile

FP32 = mybir.dt.float32
FP32R = mybir.dt.float32r
BF16 = mybir.dt.bfloat16
U32 = mybir.dt.uint32
I32 = mybir.dt.int32

NEG = -1.0e30
ID = mybir.ActivationFunctionType.Identity
EXP = mybir.ActivationFunctionType.Exp
RELU = mybir.ActivationFunctionType.Relu


def _p(**kw):
    class P:
        pass

    p = P()
    for k, v in kw.items():
        setattr(p, k, v)
    return p


def make_params(B=512, D=256, N=200000, K=128, ncores=8, chunk=1024):
    nshard = N // ncores
    nch = math.ceil(nshard / chunk)
    return _p(
        B=B, D=D, N=N, K=K, ncores=ncores, chunk=chunk,
        nshard=nshard, nch=nch, npad=nch * chunk, qpc=B // ncores,
        cand=nch * 8, mw=ncores * nch * 8,
        nqb=max(B // 128, 1), qb_size=min(B, 128),
        nheads=8, hd=D // 8, dh=D // 2,
    )


def fp32r_round(a):
    """Host-side float32r rounding (round-to-nearest, 11-bit mantissa)."""
    b = np.ascontiguousarray(a, np.float32).view(np.uint32)
    r = (b + np.uint32(0x800)) & np.uint32(0xFFFFF000)
    return r.view(np.float32)


# ---------------------------------------------------------------------------
# Builder
# ---------------------------------------------------------------------------

def build_nc(p):
    nc = bacc.Bacc("TRN2", target_bir_lowering=False, debug=False,
                   num_devices=p.ncores)
    dt = FP32
    KD = 2
    QB, NQB = p.qb_size, p.nqb
    CH, NCH, CAND = p.chunk, p.nch, p.cand
    QPC, MW = p.qpc, p.mw
    ROWW = 2 * CAND

    def din(name, shape, dtype=dt):
        return nc.dram_tensor(name, shape, dtype, kind="ExternalInput")

    qT = din("qT", [128, KD, p.B], FP32R)
    qT_mine = din("qT_mine", [128, KD, QPC])
    eT = din("eT", [128, KD, p.npad], FP32R)
    eFull = din("eFull", [p.N, p.D])
    base_add = din("base_add", [128, CAND], U32)
    qbaseT = din("qbaseT", [128, QPC])       # i*CAND
    qiota = din("qiota", [128, QPC])         # i
    hmask = din("hmask", [128, KD, 8])
    ident = din("ident", [128, 128])
    Wcd = din("Wcd", [128, KD, KD, 128])
    Wk = din("Wk", [128, KD, KD, 128])       # M_k needs untransposed chunks
    WkT = din("WkT", [128, KD, p.D])
    WvT = din("WvT", [128, KD, p.D])
    WcqT = din("WcqT", [128, KD, p.D])
    WqT = din("WqT", [128, KD, p.D])
    WoT = din("WoT", [128, KD, p.D])
    Ws1T = din("Ws1T", [128, KD, p.dh])
    Ws2T = din("Ws2T", [128, 1])
    b_cq = din("b_cq", [128, KD, 1])
    b_cd = din("b_cd", [128, KD, 1])
    b_q = din("b_q", [128, KD, 1])
    b_k = din("b_k", [128, KD, 1])
    b_v = din("b_v", [128, KD, 1])
    b_o = din("b_o", [128, KD, 1])
    b_s1 = din("b_s1", [p.dh, 1])
    b_s2 = din("b_s2", [1, 1])

    out_idx = nc.dram_tensor("top_idx", [QPC, p.K], I32, kind="ExternalOutput")
    out_scores = nc.dram_tensor("top_scores", [QPC, p.K], dt,
                                kind="ExternalOutput")
    out_rr = nc.dram_tensor("rerank", [1, QPC], dt, kind="ExternalOutput")

    a2a_in = nc.dram_tensor("a2a_in", [p.B, ROWW], dt)
    a2a_out = nc.dram_tensor("a2a_out", [p.B, ROWW], dt)
    idx_scratch = nc.dram_tensor("idx_scratch", [p.K * QPC, 64], U32)

    with tile.TileContext(nc) as tc:
        # ================= Stage A: scoring + per-chunk top-8 =============
        with tc.tile_pool(name="qt", bufs=1) as qt_pool, \
             tc.tile_pool(name="et", bufs=3) as et_pool, \
             tc.tile_pool(name="psA", bufs=4, space="PSUM") as psA, \
             tc.tile_pool(name="candp", bufs=1) as cand_pool, \
             tc.tile_pool(name="miscA", bufs=1) as miscA:

            qT_sb = qt_pool.tile([128, KD, p.B], FP32R)
            nc.sync.dma_start(out=qT_sb[:], in_=qT[:])
            base_sb = miscA.tile([128, CAND], U32)
            nc.sync.dma_start(out=base_sb[:], in_=base_add[:])

            packs = [cand_pool.tile([QB, ROWW], U32, tag=f"pack{qb}",
                                    name=f"pack{qb}")
                     for qb in range(NQB)]

            for c in range(NCH):
                et_tile = et_pool.tile([128, KD, CH], FP32R, tag="et")
                nc.sync.dma_start(out=et_tile[:],
                                  in_=eT[:, :, c * CH:(c + 1) * CH])
                for qb in range(NQB):
                    ps = psA.tile([128, CH], FP32, tag="psA")
                    mmw = min(CH, 512)
                    for h in range(CH // mmw):
                        for kc in range(KD):
                            nc.tensor.matmul(
                                out=ps[:QB, h * mmw:(h + 1) * mmw],
                                lhsT=qT_sb[:, kc, qb * QB:(qb + 1) * QB],
                                rhs=et_tile[:, kc, h * mmw:(h + 1) * mmw],
                                start=(kc == 0), stop=(kc == KD - 1))
                    pv = packs[qb][:].rearrange("q (c t) -> q c t", t=2)
                    vals8 = pv[:, c * 8:(c + 1) * 8, 0].bitcast(FP32)
                    nc.vector.max(out=vals8, in_=ps[:QB, :])
                    nc.vector.max_index(
                        out=pv[:, c * 8:(c + 1) * 8, 1],
                        in_max=vals8, in_values=ps[:QB, :])
            for qb in range(NQB):
                pv = packs[qb][:].rearrange("q (c t) -> q c t", t=2)
                nc.vector.tensor_tensor(
                    out=pv[:, :, 1], in0=pv[:, :, 1],
                    in1=base_sb[:QB, :], op=mybir.AluOpType.add)
                nc.sync.dma_start(
                    out=a2a_in[qb * QB:(qb + 1) * QB, :].bitcast(U32),
                    in_=packs[qb][:])

            nc.gpsimd.collective_compute(
                "AllToAll", mybir.AluOpType.bypass,
                replica_groups=[list(range(p.ncores))],
                ins=[a2a_in[:].opt()], outs=[a2a_out[:].opt()])

        # ================= Stage C: global merge ==========================
        with tc.tile_pool(name="mrg", bufs=1) as mrg_pool, \
             tc.tile_pool(name="sel", bufs=1) as sel_pool, \
             tc.tile_pool(name="pst", bufs=1, space="PSUM") as pst:
            # contiguous load of all (val, idx) pairs for my queries;
            # the extraction scans the stride-2 value view in place.
            pairs_sb = mrg_pool.tile([QPC, p.ncores, 2 * CAND], dt)
            nc.sync.dma_start(
                out=pairs_sb[:],
                in_=a2a_out.ap().rearrange("(s i) w -> i s w", s=p.ncores))
            vals_sb = mrg_pool.tile([QPC, MW], dt)
            nc.vector.tensor_copy(
                out=vals_sb[:].rearrange("i (s w) -> i s w", s=p.ncores),
                in_=pairs_sb[:].rearrange("i s (w t) -> i s w t",
                                          t=2)[:, :, :, 0])
            work = vals_sb[:]

            appr_sb = sel_pool.tile([QPC, p.K], dt)
            slots_sb = sel_pool.tile([QPC, p.K], U32)
            nrounds = p.K // 8
            for r in range(nrounds):
                v8 = appr_sb[:, r * 8:(r + 1) * 8]
                nc.vector.max(out=v8, in_=work)
                nc.vector.max_index(out=slots_sb[:, r * 8:(r + 1) * 8],
                                    in_max=v8, in_values=work)
                if r != nrounds - 1:
                    nc.vector.match_replace(out=work, in_to_replace=v8,
                                            in_values=work, imm_value=NEG)

            # ---- slot -> pair index (rank-on-partition orientation) ----
            inv = np.float32(1.0) / np.float32(CAND)
            MAGIC = np.float32(12582912.0)
            FEPS = np.float32(0.4985)
            fl = np.float32(np.float32(np.float32(
                np.arange(MW, dtype=np.float32) * inv) - FEPS) + MAGIC) - MAGIC
            assert np.all(fl == (np.arange(MW) // CAND)), "floor trick invalid"

            id_sb = sel_pool.tile([128, 128], dt)
            nc.sync.dma_start(out=id_sb[:], in_=ident[:])
            qb_sb = sel_pool.tile([128, QPC], dt)
            nc.sync.dma_start(out=qb_sb[:], in_=qbaseT[:])
            sf = sel_pool.tile([QPC, p.K], dt)
            nc.vector.tensor_copy(out=sf[:], in_=slots_sb[:])
            tp = pst.tile([128, 128], FP32, tag="tp")
            nc.tensor.transpose(out=tp[:, :QPC], in_=sf[:],
                                identity=id_sb[:QPC, :QPC])
            sfT = sel_pool.tile([p.K, QPC], dt)
            nc.scalar.activation(out=sfT[:], in_=tp[:p.K, :QPC], func=ID)
            segf = sel_pool.tile([p.K, QPC], dt)
            nc.vector.tensor_scalar(out=segf[:], in0=sfT[:],
                                    scalar1=float(inv), scalar2=float(FEPS),
                                    op0=mybir.AluOpType.mult,
                                    op1=mybir.AluOpType.subtract)
            nc.vector.tensor_scalar(out=segf[:], in0=segf[:],
                                    scalar1=float(MAGIC), scalar2=float(MAGIC),
                                    op0=mybir.AluOpType.add,
                                    op1=mybir.AluOpType.subtract)
            nc.vector.tensor_scalar(out=segf[:], in0=segf[:],
                                    scalar1=float(CAND * (QPC - 1)),
                                    scalar2=None, op0=mybir.AluOpType.mult)
            nc.vector.tensor_add(sfT[:], sfT[:], segf[:])
            nc.vector.tensor_tensor(out=sfT[:], in0=sfT[:],
                                    in1=qb_sb[:p.K, :],
                                    op=mybir.AluOpType.add)
            flat_u = sel_pool.tile([p.K, QPC], U32)
            nc.vector.tensor_copy(out=flat_u[:], in_=sfT[:])

            # ---- per-query gathers: (val, idx) pair then embedding row ----
            a2a_pairs = a2a_out.ap().rearrange("b (w t) -> (b w) t", t=2)
            pairbuf = sel_pool.tile([p.K, QPC, 2], dt)
            cand_all = sel_pool.tile([p.K, QPC, p.D], dt)
            for i in range(QPC):
                nc.gpsimd.indirect_dma_start(
                    out=pairbuf[:, i, :], out_offset=None,
                    in_=a2a_pairs,
                    in_offset=bass.IndirectOffsetOnAxis(
                        ap=flat_u[:, i:i + 1], axis=0))
                nc.gpsimd.indirect_dma_start(
                    out=cand_all[:, i, :], out_offset=None,
                    in_=eFull[:],
                    in_offset=bass.IndirectOffsetOnAxis(
                        ap=pairbuf[:, i, 1:2].bitcast(U32), axis=0))

            # stash gathered ids for the final permutation (col 0 of each
            # 256-byte row so dma_gather's row-size constraint is met)
            nc.sync.dma_start(
                out=idx_scratch[:, 0:1],
                in_=pairbuf[:, :, 1].bitcast(U32).unsqueeze(2))

            _rerank(nc, tc, p, cand_all, pairbuf, idx_scratch, qT_mine,
                    WcqT, WqT, Wcd, Wk, WkT, WvT, WoT, Ws1T, Ws2T, b_cq,
                    b_cd, b_q, b_k, b_v, b_o, b_s1, b_s2, hmask, qiota,
                    id_sb, pst, out_idx, out_scores, out_rr)

    nc.compile()
    return nc


def _rerank(nc, tc, p, cand_all, pairbuf, idx_scratch, qT_mine, WcqT, WqT,
            Wcd, Wk, WkT, WvT, WoT, Ws1T, Ws2T, b_cq, b_cd, b_q, b_k, b_v,
            b_o, b_s1, b_s2, hmask, qiota, id_sb, pst, out_idx, out_scores,
            out_rr):
    dt = FP32
    KD = 2
    QPC = p.qpc

    with tc.tile_pool(name="wts", bufs=1) as wp, \
         tc.tile_pool(name="cT", bufs=6) as cT_pool, \
         tc.tile_pool(name="kvV", bufs=17) as kvV, \
         tc.tile_pool(name="at", bufs=4) as at_pool, \
         tc.tile_pool(name="ctxp", bufs=1) as ctx_pool, \
         tc.tile_pool(name="psB", bufs=3, space="PSUM") as psB, \
         tc.tile_pool(name="psSm", bufs=2, space="PSUM") as psSm:

        def big_ps():
            return psB.tile([128, 512], FP32, tag="big", name="bigps")

        def ldb(t, shape):
            """Load a weight tensor, casting fp32 -> bf16 in the DMA."""
            s = wp.tile(shape, BF16, tag=t.name, name="w_" + t.name)
            nc.gpsimd.dma_start(out=s[:], in_=t[:])
            return s

        def ldf(t, shape):
            s = wp.tile(shape, dt, tag=t.name, name="w_" + t.name)
            nc.sync.dma_start(out=s[:], in_=t[:])
            return s

        wcd = ldb(Wcd, [128, KD, KD, 128])
        wkT = ldb(WkT, [128, KD, p.D])
        wvT = ldb(WvT, [128, KD, p.D])
        wcqT = ldb(WcqT, [128, KD, p.D])
        wqT = ldb(WqT, [128, KD, p.D])
        woT = ldb(WoT, [128, KD, p.D])
        ws1T = ldb(Ws1T, [128, KD, p.dh])
        ws2T = ldb(Ws2T, [128, 1])
        bcq = ldf(b_cq, [128, KD, 1])
        bcd = ldf(b_cd, [128, KD, 1])
        bq = ldf(b_q, [128, KD, 1])
        bk = ldf(b_k, [128, KD, 1])
        bv = ldf(b_v, [128, KD, 1])
        bo = ldf(b_o, [128, KD, 1])
        bs1 = ldf(b_s1, [p.dh, 1])
        bs2 = ldf(b_s2, [1, 1])
        hm = ldb(hmask, [128, KD, 8])
        hmf = ldf(hmask, [128, KD, 8])
        qtm = ldf(qT_mine, [128, KD, QPC])
        qtm_b = wp.tile([128, KD, QPC], BF16, tag="qtmb")
        nc.vector.tensor_copy(out=qtm_b[:], in_=qtm[:])
        qio = ldf(qiota, [128, QPC])

        # M_vT (bf16): [128(j), jc, i] = (Wv @ Wcd)[i, jc*128+j]
        mvT = wp.tile([128, KD, p.D], BF16, tag="mvT")
        for jc in range(KD):
            ps = big_ps()
            for lc in range(KD):
                nc.tensor.matmul(out=ps[:, :p.D], lhsT=wcd[:, lc, jc, :],
                                 rhs=wvT[:, lc, :],
                                 start=(lc == 0), stop=(lc == KD - 1))
            nc.scalar.activation(out=mvT[:, jc, :], in_=ps[:, :p.D], func=ID)
        # M_k untransposed chunks (bf16): mk[128(j), jc(j), ic(j'), 128(j')]
        #   = M_k[jc*128+j, ic*128+j'] = sum_l Wk[j, l] Wcd[l, j']
        mk = wp.tile([128, KD, KD, 128], BF16, tag="mk")
        for jc in range(KD):
            ps = big_ps()
            for lc in range(KD):
                # out[j, j'] = sum_l WkT[l, jc*128+j]^T ... lhsT = wkT chunk
                nc.tensor.matmul(out=ps[:, :p.D],
                                 lhsT=wkT[:, lc, jc * 128:(jc + 1) * 128],
                                 rhs=wcd[:, lc, :, :].rearrange("l a b -> l (a b)"),
                                 start=(lc == 0), stop=(lc == KD - 1))
            nc.scalar.activation(
                out=mk[:, jc, :, :].rearrange("j a b -> j (a b)"),
                in_=ps[:, :p.D], func=ID)

        # c_k = Wk @ b_cd + bk ; c_v = Wv @ b_cd + bv
        bcd_b = wp.tile([128, KD, 1], BF16, tag="bcdb")
        nc.vector.tensor_copy(out=bcd_b[:], in_=bcd[:])
        ck = wp.tile([128, KD, 1], dt, tag="ck")
        cv = wp.tile([128, KD, 1], dt, tag="cv")
        for (ct, wt, bt) in ((ck, wkT, bk), (cv, wvT, bv)):
            for m in range(KD):
                ps = psSm.tile([128, 8], FP32, tag="small", name="smps")
                for jc in range(KD):
                    nc.tensor.matmul(out=ps[:, :1],
                                     lhsT=wt[:, jc, m * 128:(m + 1) * 128],
                                     rhs=bcd_b[:, jc, :],
                                     start=(jc == 0), stop=(jc == KD - 1))
                nc.scalar.activation(out=ct[:, m, :], in_=ps[:, :1], func=ID,
                                     bias=bt[:, m, :])

        # Q'T = W_cq @ qT_mine + b_cq ; qhT = (Wq @ Q'T + bq) / sqrt(hd)
        qpT = wp.tile([128, KD, QPC], BF16, tag="qpT")
        qhT = wp.tile([128, KD, QPC], dt, tag="qhT")
        for m in range(KD):
            ps = big_ps()
            for jc in range(KD):
                nc.tensor.matmul(out=ps[:, :QPC],
                                 lhsT=wcqT[:, jc, m * 128:(m + 1) * 128],
                                 rhs=qtm_b[:, jc, :], start=(jc == 0),
                                 stop=(jc == KD - 1))
            nc.scalar.activation(out=qpT[:, m, :], in_=ps[:, :QPC], func=ID,
                                 bias=bcq[:, m, :])
        sc = 1.0 / math.sqrt(p.hd)
        bq_s = wp.tile([128, KD, 1], dt, tag="bqs")
        nc.vector.tensor_scalar(out=bq_s[:], in0=bq[:], scalar1=sc,
                                scalar2=None, op0=mybir.AluOpType.mult)
        for m in range(KD):
            ps = big_ps()
            for jc in range(KD):
                nc.tensor.matmul(out=ps[:, :QPC],
                                 lhsT=wqT[:, jc, m * 128:(m + 1) * 128],
                                 rhs=qpT[:, jc, :], start=(jc == 0),
                                 stop=(jc == KD - 1))
            nc.scalar.activation(out=qhT[:, m, :], in_=ps[:, :QPC], func=ID,
                                 bias=bq_s[:, m, :], scale=sc)

        # QhS_all[j, jc, q, h] = qhT * hmask ; G = M_k^T @ QhS (bf16)
        qhs_all = wp.tile([128, KD, QPC, 8], BF16, tag="qhs")
        nc.vector.tensor_tensor(
            out=qhs_all[:],
            in0=qhT[:].unsqueeze(3).to_broadcast([128, KD, QPC, 8]),
            in1=hmf[:].unsqueeze(2).to_broadcast([128, KD, QPC, 8]),
            op=mybir.AluOpType.mult)
        g_all = wp.tile([128, KD, QPC, 8], BF16, tag="g_all")
        for jpc in range(KD):
            nmm = (QPC * 8) // 512
            for b in range(max(nmm, 1)):
                w = min(512, QPC * 8)
                ps = big_ps()
                for jc in range(KD):
                    nc.tensor.matmul(
                        out=ps[:, :w],
                        lhsT=mk[:, jc, jpc, :],
                        rhs=qhs_all[:, jc, :, :].rearrange(
                            "j q h -> j (q h)")[:, b * 512:b * 512 + w],
                        start=(jc == 0), stop=(jc == KD - 1))
                nc.scalar.activation(
                    out=g_all[:, jpc, :, :].rearrange(
                        "j q h -> j (q h)")[:, b * 512:b * 512 + w],
                    in_=ps[:, :w], func=ID)

        # ---- per-query: transpose cand, rescore, attention ----
        sexT = ctx_pool.tile([p.K, QPC], dt)
        ctxcol = ctx_pool.tile([128, KD, QPC, 8], dt)
        nbat = max(QPC // 16, 1)
        bsz = QPC // nbat
        for bi in range(nbat):
            attn = at_pool.tile([8, bsz, 128], dt, tag="attn")
            vhs = []
            for qq in range(bsz):
                q = bi * bsz + qq
                candq = cand_all[:, q, :]
                cTf = cT_pool.tile([128, KD, 128], dt, tag="cTf")
                cTb = cT_pool.tile([128, KD, 128], BF16, tag="cTb")
                for m in range(KD):
                    tpp = big_ps()
                    nc.tensor.transpose(out=tpp[:, :128],
                                        in_=candq[:, m * 128:(m + 1) * 128],
                                        identity=id_sb[:])
                    nc.scalar.activation(out=cTf[:, m, :], in_=tpp[:, :128],
                                         func=ID)
                    nc.scalar.activation(out=cTb[:, m, :], in_=tpp[:, :128],
                                         func=ID)
                # exact rescore: s_ex[k] = sum_d candT[d,k] * q[d]
                ps = psSm.tile([128, 8], FP32, tag="small", name="smps2")
                for jc in range(KD):
                    nc.tensor.matmul(out=ps[:, :1], lhsT=cTf[:, jc, :],
                                     rhs=qtm[:, jc, q:q + 1],
                                     start=(jc == 0), stop=(jc == KD - 1))
                nc.scalar.activation(out=sexT[:, q:q + 1], in_=ps[:, :1],
                                     func=ID)
                # vh = cand @ M_v^T + (c_v deferred)  [128k, D] bf16
                vh = kvV.tile([128, p.D], BF16, tag="vh")
                ps2 = big_ps()
                for jc in range(KD):
                    nc.tensor.matmul(out=ps2[:, :p.D], lhsT=cTb[:, jc, :],
                                     rhs=mvT[:, jc, :],
                                     start=(jc == 0), stop=(jc == KD - 1))
                nc.scalar.activation(out=vh[:], in_=ps2[:, :p.D], func=ID)
                vhs.append(vh)
                # logits = G_q^T @ candT
                lg_ps = psSm.tile([8, 128], FP32, tag="lgps", name="lgps")
                for jpc in range(KD):
                    nc.tensor.matmul(out=lg_ps[:],
                                     lhsT=g_all[:, jpc, q, :],
                                     rhs=cTb[:, jpc, :],
                                     start=(jpc == 0), stop=(jpc == KD - 1))
                nc.scalar.activation(out=attn[:, qq, :], in_=lg_ps[:],
                                     func=ID)
            nmax = at_pool.tile([8, bsz], dt, tag="nmax")
            nc.vector.tensor_reduce(out=nmax[:], in_=attn[:],
                                    axis=mybir.AxisListType.X,
                                    op=mybir.AluOpType.max, negate=True)
            nc.vector.tensor_tensor(
                out=attn[:], in0=attn[:],
                in1=nmax[:].unsqueeze(2).to_broadcast([8, bsz, 128]),
                op=mybir.AluOpType.add)
            nc.scalar.activation(out=attn[:], in_=attn[:], func=EXP)
            ssum = at_pool.tile([8, bsz], dt, tag="ssum")
            nc.vector.tensor_reduce(out=ssum[:], in_=attn[:],
                                    axis=mybir.AxisListType.X,
                                    op=mybir.AluOpType.add)
            rinv = at_pool.tile([8, bsz], dt, tag="rinv")
            nc.vector.reciprocal(out=rinv[:], in_=ssum[:])
            nc.vector.tensor_tensor(
                out=attn[:], in0=attn[:],
                in1=rinv[:].unsqueeze(2).to_broadcast([8, bsz, 128]),
                op=mybir.AluOpType.mult)
            for qq in range(bsz):
                q = bi * bsz + qq
                tpp = psSm.tile([128, 8], FP32, tag="small", name="smps3")
                nc.tensor.transpose(out=tpp[:], in_=attn[:, qq, :],
                                    identity=id_sb[:8, :8])
                attnT = at_pool.tile([128, 8], BF16, tag="attnT")
                nc.scalar.activation(out=attnT[:], in_=tpp[:], func=ID)
                for m in range(KD):
                    ps3 = psSm.tile([128, 8], FP32, tag="small", name="smps4")
                    nc.tensor.matmul(out=ps3[:],
                                     lhsT=vhs[qq][:, m * 128:(m + 1) * 128],
                                     rhs=attnT[:], start=True, stop=True)
                    nc.scalar.activation(out=ctxcol[:, m, q, :], in_=ps3[:],
                                         func=ID, bias=cv[:, m, :])

        # ---- exact resort ----
        tp = pst.tile([128, 128], FP32, tag="tp")
        nc.tensor.transpose(out=tp[:QPC, :p.K], in_=sexT[:],
                            identity=id_sb[:])
        sex = ctx_pool.tile([QPC, p.K], dt)
        nc.scalar.activation(out=sex[:], in_=tp[:QPC, :p.K], func=ID)
        swork = ctx_pool.tile([QPC, p.K], dt)
        nc.vector.tensor_copy(out=swork[:], in_=sex[:])
        scr_sb = ctx_pool.tile([QPC, p.K], dt)
        slots3 = ctx_pool.tile([QPC, p.K], U32)
        nr = p.K // 8
        for r in range(nr):
            v8 = scr_sb[:, r * 8:(r + 1) * 8]
            nc.vector.max(out=v8, in_=swork[:])
            nc.vector.max_index(out=slots3[:, r * 8:(r + 1) * 8],
                                in_max=v8, in_values=swork[:])
            if r != nr - 1:
                nc.vector.match_replace(out=swork[:], in_to_replace=v8,
                                        in_values=swork[:], imm_value=NEG)
        nc.sync.dma_start(out=out_scores[:], in_=scr_sb[:])

        # ---- permute ids by exact rank via DRAM scratch ----
        # pipelined by rank halves: the dma_gather for ranks [0,64) runs
        # while the resort rounds for ranks [64,128) still execute.
        for hh in range(2):
            j0 = hh * (p.K // 2)
            s3f = ctx_pool.tile([QPC, p.K // 2], dt, tag="s3f", bufs=2,
                                name=f"s3f{hh}")
            nc.vector.tensor_copy(out=s3f[:],
                                  in_=slots3[:, j0:j0 + p.K // 2])
            tp2 = pst.tile([128, 128], FP32, tag="tp", name=f"tp2{hh}")
            nc.tensor.transpose(out=tp2[:p.K // 2, :QPC], in_=s3f[:],
                                identity=id_sb[:QPC, :QPC])
            f3T = ctx_pool.tile([p.K // 2, QPC], dt, tag="f3T", bufs=2,
                                name=f"f3T{hh}")
            nc.scalar.activation(out=f3T[:], in_=tp2[:p.K // 2, :QPC],
                                 func=ID, scale=float(QPC))
            nc.vector.tensor_tensor(out=f3T[:], in0=f3T[:],
                                    in1=qio[:p.K // 2, :],
                                    op=mybir.AluOpType.add)
            f3i = ctx_pool.tile([p.K // 2, QPC], mybir.dt.int16, tag="f3i", bufs=2,
                                name=f"f3i{hh}")
            nc.vector.tensor_copy(out=f3i[:], in_=f3T[:])
            # idx list order k = q*(K/2) + j -> wrapped[(j%16), q*(K/32)+j//16]
            nh = p.K // 32          # hi values per half (4 for K=128)
            wrapped = ctx_pool.tile([128, QPC * nh], mybir.dt.int16,
                                    tag="wrap", bufs=2, name=f"wrap{hh}")
            wv = wrapped[:].rearrange("pp (q h) -> pp q h", h=nh)
            for hi in range(nh):
                nc.sync.dma_start(out=wv[:16, :, hi],
                                  in_=f3i[hi * 16:(hi + 1) * 16, :])
            for g in range(1, 8):
                nc.sync.dma_start(out=wrapped[g * 16:(g + 1) * 16, :],
                                  in_=wrapped[:16, :])
            nidx = (p.K // 2) * QPC
            gbuf = ctx_pool.tile([128, nidx // 128, 64], dt, tag="gbuf", bufs=2,
                                 name=f"gbuf{hh}")
            nc.gpsimd.dma_gather(
                out_ap=gbuf[:], in_ap=idx_scratch.ap().bitcast(FP32),
                idxs_ap=wrapped[:],
                num_idxs=nidx, num_idxs_reg=nidx, elem_size=64,
                single_packet=False)
            # out[p, b] = idx(q = 2b + p//64, j = p%64): transpose, then
            # rows b hold [q=2b ranks | q=2b+1 ranks]
            gf = ctx_pool.tile([128, nidx // 128], dt, tag="gf", bufs=2,
                               name=f"gf{hh}")
            nc.vector.tensor_copy(out=gf[:], in_=gbuf[:, :, 0].bitcast(U32))
            tp4 = pst.tile([128, 128], FP32, tag="tp", name=f"tp4{hh}")
            nc.tensor.transpose(out=tp4[:nidx // 128, :128], in_=gf[:],
                                identity=id_sb[:])
            tsb = ctx_pool.tile([nidx // 128, 128], I32, tag="tsb", bufs=2,
                                name=f"tsb{hh}")
            nc.vector.tensor_copy(out=tsb[:], in_=tp4[:nidx // 128, :128])
            nc.sync.dma_start(
                out=out_idx.ap()[:, j0:j0 + p.K // 2].rearrange(
                    "(b par) j -> b par j", par=2),
                in_=tsb[:].rearrange("b (par j) -> b par j", par=2))

        # ---- diag extract + output head ----
        ctxTf = ctx_pool.tile([128, KD, QPC], dt)
        tmp = ctx_pool.tile([128, KD, QPC, 8], dt)
        nc.vector.tensor_tensor(
            out=tmp[:], in0=ctxcol[:],
            in1=hmf[:].unsqueeze(2).to_broadcast([128, KD, QPC, 8]),
            op=mybir.AluOpType.mult)
        nc.vector.tensor_reduce(out=ctxTf[:], in_=tmp[:],
                                axis=mybir.AxisListType.X,
                                op=mybir.AluOpType.add)
        ctxT = ctx_pool.tile([128, KD, QPC], BF16)
        nc.vector.tensor_copy(out=ctxT[:], in_=ctxTf[:])

        crossT = ctx_pool.tile([128, KD, QPC], BF16)
        for m in range(KD):
            ps = big_ps()
            for jc in range(KD):
                nc.tensor.matmul(out=ps[:, :QPC],
                                 lhsT=woT[:, jc, m * 128:(m + 1) * 128],
                                 rhs=ctxT[:, jc, :], start=(jc == 0),
                                 stop=(jc == KD - 1))
            nc.scalar.activation(out=crossT[:, m, :], in_=ps[:, :QPC],
                                 func=ID, bias=bo[:, m, :])
        s1T = ctx_pool.tile([p.dh, QPC], BF16)
        ps = big_ps()
        for jc in range(KD):
            nc.tensor.matmul(out=ps[:p.dh, :QPC], lhsT=ws1T[:, jc, :],
                             rhs=crossT[:, jc, :], start=(jc == 0),
                             stop=(jc == KD - 1))
        nc.scalar.activation(out=s1T[:], in_=ps[:p.dh, :QPC], func=RELU,
                             bias=bs1[:])
        rr_ps = big_ps()
        nc.tensor.matmul(out=rr_ps[:1, :QPC], lhsT=ws2T[:p.dh, :],
                         rhs=s1T[:], start=True, stop=True)
        rr_sb = ctx_pool.tile([1, QPC], dt)
        nc.scalar.activation(out=rr_sb[:], in_=rr_ps[:1, :QPC], func=ID,
                             bias=bs2[:])
        nc.sync.dma_start(out=out_rr[:], in_=rr_sb[:])


# ---------------------------------------------------------------------------
# Host-side glue
# ---------------------------------------------------------------------------

def _to2chunk(w):
    return np.ascontiguousarray(w.reshape(2, 128, -1).transpose(1, 0, 2))


def prepare_in_maps(p, inputs):
    f32 = np.float32
    q = np.asarray(inputs["query_emb"], f32)
    E = np.asarray(inputs["index_embs"], f32)
    W_cq = np.asarray(inputs["W_cq"], f32)
    b_cq = np.asarray(inputs["b_cq"], f32)
    W_cd = np.asarray(inputs["W_cd"], f32)
    b_cd = np.asarray(inputs["b_cd"], f32)
    ipw = np.asarray(inputs["in_proj_w"], f32)
    ipb = np.asarray(inputs["in_proj_b"], f32)
    Wq, Wk, Wv = ipw[:p.D], ipw[p.D:2 * p.D], ipw[2 * p.D:]
    bq, bk, bv = ipb[:p.D], ipb[p.D:2 * p.D], ipb[2 * p.D:]
    W_o = np.asarray(inputs["out_proj_w"], f32)
    b_o = np.asarray(inputs["out_proj_b"], f32)
    W_s1 = np.asarray(inputs["W_s1"], f32)
    b_s1 = np.asarray(inputs["b_s1"], f32)
    W_s2 = np.asarray(inputs["W_s2"], f32)
    b_s2 = np.asarray(inputs["b_s2"], f32)

    qT = _to2chunk(np.ascontiguousarray(q.T))
    heads = (np.arange(p.D) // p.hd)
    hmask = np.zeros((p.D, 8), f32)
    hmask[np.arange(p.D), heads] = 1.0
    hmask = _to2chunk(hmask)
    ident = np.eye(128, dtype=f32)
    wcd = np.ascontiguousarray(
        W_cd.reshape(2, 128, 2, 128).transpose(1, 0, 2, 3))
    wk = np.ascontiguousarray(
        Wk.reshape(2, 128, 2, 128).transpose(1, 0, 2, 3))

    def T2(w):
        return _to2chunk(np.ascontiguousarray(w.T))

    def bvec(b):
        return np.ascontiguousarray(b.reshape(2, 128, 1).transpose(1, 0, 2))

    common = dict(
        qT=fp32r_round(qT), eFull=E, hmask=hmask, ident=ident,
        Wcd=wcd, Wk=wk, WkT=T2(Wk), WvT=T2(Wv), WcqT=T2(W_cq), WqT=T2(Wq),
        WoT=T2(W_o), Ws1T=T2(W_s1), Ws2T=np.ascontiguousarray(W_s2.T),
        b_cq=bvec(b_cq), b_cd=bvec(b_cd), b_q=bvec(bq), b_k=bvec(bk),
        b_v=bvec(bv), b_o=bvec(b_o),
        b_s1=np.ascontiguousarray(b_s1.reshape(p.dh, 1)),
        b_s2=np.ascontiguousarray(b_s2.reshape(1, 1)),
    )
    in_maps = []
    for c in range(p.ncores):
        esh = E[c * p.nshard:(c + 1) * p.nshard]
        eTc = np.zeros((p.D, p.npad), f32)
        eTc[:, :p.nshard] = esh.T
        base = np.zeros((128, p.cand), np.uint32)
        for ch in range(p.nch):
            base[:, ch * 8:(ch + 1) * 8] = c * p.nshard + ch * p.chunk
        qbase = np.broadcast_to(
            (np.arange(p.qpc, dtype=np.float64) * p.cand).astype(f32),
            (128, p.qpc)).copy()
        qio = np.broadcast_to(
            np.arange(p.qpc, dtype=f32), (128, p.qpc)).copy()
        m = dict(common)
        m["eT"] = fp32r_round(_to2chunk(eTc))
        m["base_add"] = base
        m["qT_mine"] = np.ascontiguousarray(qT[:, :, c * p.qpc:(c + 1) * p.qpc])
        m["qbaseT"] = qbase
        m["qiota"] = qio
        in_maps.append(m)
    return in_maps


_CACHE = {}


def _get_nc(p):
    key = (p.B, p.N, p.chunk)
    if key not in _CACHE:
        _CACHE[key] = build_nc(p)
    return _CACHE[key]


def run(inputs, trace=False, **kw):
    from concourse.bass_utils import run_bass_kernel_spmd
    p = make_params()
    nc = _get_nc(p)
    in_maps = prepare_in_maps(p, inputs)
    res = run_bass_kernel_spmd(nc, in_maps, core_ids=list(range(p.ncores)),
                               trace=trace, **kw)
    outs = res.results
    top_idx = np.concatenate([o["top_idx"] for o in outs], axis=0)
    top_scores = np.concatenate([o["top_scores"] for o in outs], axis=0)
    rerank = np.concatenate([o["rerank"].reshape(-1) for o in outs], axis=0)
    return (top_idx.astype(np.int32), top_scores, rerank), res


def kernel(**inputs):
    out, _ = run(inputs, trace=False)
    return out
